# revision 1
# baseline (speedup 1.0000x reference)
"""Trainium2 Bass kernel: multi-head flash self-attention with RoPE.

Problem: x[4,2048,1024], 16 heads, dh=64, causal, RoPE(theta=10000), WO proj.

Sharding (8 cores): core c -> batch b=c//2, head-group g=c%2 (8 heads each).
Per core:
  - QKV projections of x[b] (bf16 matmuls, fp32 PSUM accumulation).
  - RoPE folded into a host-side weight-row permutation (per head: even dims
    then odd dims) so the rotation becomes tile-local partition algebra.
  - Flash attention in S^T layout ([k,q] blocks). V is stored per (ktile,
    head-pair) as [V_A | ones | V_B] so each head's stationary operand is a
    contiguous 128 cols and the softmax denominators appear as 64 replicated
    PSUM rows. No max subtraction (scores ~ N(0,1) by construction).
  - Pairwise AllGather of normalized O^T; each core then computes the output
    projection for ALL 2048 rows but only its 512 WO columns (keeps the SPMD
    program identical across cores).
Host reassembles: out[b] = concat(cols of core 2b, cols of core 2b+1).
"""
import sys

sys.path.insert(0, "/opt/trn_rl_repo")

import numpy as np
import ml_dtypes
import concourse.bass as bass
import concourse.bacc as bacc
import concourse.mybir as mybir
from concourse import tile
from concourse.bass_utils import run_bass_kernel_spmd

f32 = mybir.dt.float32
bf16 = mybir.dt.bfloat16
AF = mybir.ActivationFunctionType

S = 2048
D = 1024
H = 16
DH = 64
NCORE = 8
SL = 512           # local m dims (8 heads x 64)
NEG = -1e30
SCALE = 1.0 / 8.0  # 1/sqrt(dh)
GROUPS = [[0, 1], [2, 3], [4, 5], [6, 7]]
VPP = 192          # v_store cols per (ktile, pair): [V_A | ones | V_B]
VKT = 4 * VPP      # v_store cols per ktile


def build(timing=False):
    nc = bacc.Bacc("TRN2", target_bir_lowering=False, debug=False,
                   num_devices=1 if timing else NCORE)

    xT = nc.dram_tensor("xT", [D, S], bf16, kind="ExternalInput").ap()
    wqT = nc.dram_tensor("wqT", [D, SL], bf16, kind="ExternalInput").ap()
    wkT = nc.dram_tensor("wkT", [D, SL], bf16, kind="ExternalInput").ap()
    wvT = nc.dram_tensor("wvT", [D, SL], bf16, kind="ExternalInput").ap()
    woT = nc.dram_tensor("woT", [D, SL], bf16, kind="ExternalInput").ap()
    cosr = nc.dram_tensor("cosr", [128, S], f32, kind="ExternalInput").ap()
    sinr = nc.dram_tensor("sinr", [128, S], f32, kind="ExternalInput").ap()
    out = nc.dram_tensor("out", [S, SL], f32, kind="ExternalOutput").ap()

    og_send = [nc.dram_tensor(f"og_send{p}", [128, S], bf16) for p in range(4)]
    acc_d = [nc.dram_tensor(f"acc_d{i}", [128, SL], f32) for i in range(16)]
    og_recv = [nc.dram_tensor(f"og_recv{p}", [256, S], bf16) for p in range(4)]

    with tile.TileContext(nc) as tc:
        _body(nc, tc, xT, wqT, wkT, wvT, woT, cosr, sinr, out,
              og_send, og_recv, acc_d, timing)
    nc.compile()
    return nc


def _body(nc, tc, xT, wqT, wkT, wvT, woT, cosr, sinr, out,
          og_send, og_recv, acc_d, timing=False):
    from contextlib import ExitStack
    ctx = ExitStack()
    with ctx:
        sb = ctx.enter_context(tc.tile_pool(name="sb", bufs=1))
        psp = ctx.enter_context(tc.tile_pool(name="psp", bufs=1, space="PSUM"))
        counter = [0]

        def til(shape, dtype, tag, bufs):
            counter[0] += 1
            return sb.tile(shape, dtype, tag=tag, bufs=bufs,
                           name=f"{tag}_{counter[0]}")

        # ---------------- RoPE tables (host-computed) ----------------
        cos_t = til([128, S], f32, "cos", 1)
        nc.sync.dma_start(cos_t[:], cosr[:])
        sin_t = til([128, S], f32, "sin", 1)
        nc.sync.dma_start(sin_t[:], sinr[:])

        # 0/1 triangle mask [128,128]: 1 where c - r >= 0 (valid)
        mask_t = til([128, 128], bf16, "mask", 1)
        nc.gpsimd.memset(mask_t[:], 1.0)
        nc.gpsimd.affine_select(
            out=mask_t[:], in_=mask_t[:], compare_op=mybir.AluOpType.is_ge,
            fill=0.0, base=0, pattern=[[1, 128]], channel_multiplier=-1,
        )

        # ---------------- input loads ----------------
        def load_w(wdram, tag="w"):
            tiles = []
            for dt in range(8):
                t = til([128, SL], bf16, tag, 8)
                nc.sync.dma_start(t[:], wdram[dt * 128:(dt + 1) * 128, :])
                tiles.append(t)
            return tiles

        wv_t = load_w(wvT, "w")
        xt = []
        for dt in range(8):
            t = til([128, S], bf16, "xt", 8)
            nc.sync.dma_start(t[:], xT[dt * 128:(dt + 1) * 128, :])
            xt.append(t)
        wq_t = load_w(wqT, "wq")
        wk_t = load_w(wkT, "wk")

        # v quarters: v_q[i] holds ktiles 4i..4i+4; per (kt, pair p) block
        # of VPP cols: [V_A | ones | V_B]
        v_q = []
        for i in range(16):
            vq = til([128, VKT], bf16, "v", 16)
            nc.gpsimd.memset(vq[:], 1.0)
            v_q.append(vq)

        def emit_v_quarter(i):
            for kt4 in range(4):
                kt = 4 * i + kt4
                ps = psp.tile([128, 512], f32, tag="proj", bufs=2)
                for dt in range(8):
                    nc.tensor.matmul(
                        ps[:],
                        xt[dt][:, kt * 128:(kt + 1) * 128],
                        wv_t[dt][:],
                        start=(dt == 0), stop=(dt == 7),
                    )
                vva = v_q[kt][:].rearrange("q (a c) -> q a c", c=64)
                psa = ps[:].rearrange("q (a c) -> q a c", c=64)
                nc.vector.tensor_copy(vva[:, 0:12:3, :], psa[:, 0:8:2, :])
                nc.vector.tensor_copy(vva[:, 2:12:3, :], psa[:, 1:8:2, :])

        def v_slice(kt, p, c0, c1):
            off = p * VPP
            return v_q[kt][:, off + c0:off + c1]

        # per-st projection + rope into a [128, 512] tile
        def proj_rope_st(wtiles, mt, st, fast=False):
            big_t = til([128, 512], bf16, "qk", 18)
            ps = psp.tile([128, 512], f32, tag="proj", bufs=2)
            for dt in range(8):
                nc.tensor.matmul(
                    ps[:],
                    wtiles[dt][:, mt * 128:(mt + 1) * 128],
                    xt[dt][:, st * 512:(st + 1) * 512],
                    start=(dt == 0), stop=(dt == 7),
                )
            cols = slice(st * 512, (st + 1) * 512)
            pre_t = til([128, 512], f32, "pre", 2)
            nc.vector.tensor_copy(pre_t[:], ps[:])
            swp = til([128, 512], f32, "swp", 2)
            for a in range(4):
                srcp = (a ^ 1) * 32
                nc.sync.dma_start(swp[a * 32:(a + 1) * 32, :],
                                  pre_t[srcp:srcp + 32, :])
            tmp = til([128, 512], f32, "tmp", 2)
            nc.vector.tensor_mul(tmp[:], pre_t[:], cos_t[:, cols])
            if fast:
                nc.vector.tensor_mul(swp[:], swp[:], sin_t[:, cols])
            else:
                nc.gpsimd.tensor_mul(swp[:], swp[:], sin_t[:, cols])
            nc.vector.tensor_add(big_t[:], tmp[:], swp[:])
            return big_t

        # -------- per pair: Q/K projection + rope + flash attention --------
        for p in range(4):
            qtr = [None] * 4
            ktr = [None] * 4
            if p > 0:
                for st in range(4):
                    qtr[st] = proj_rope_st(wq_t, p, st)
                for st in range(4):
                    ktr[st] = proj_rope_st(wk_t, p, st)

            for qb in range(4):
                if p == 0:
                    emit_v_quarter(qb)
                    qtr[qb] = proj_rope_st(wq_t, 0, qb, fast=(qb == 0))
                    ktr[qb] = proj_rope_st(wk_t, 0, qb, fast=(qb == 0))
                qcols_t = qtr[qb]
                oA = psp.tile([128, 512], f32, tag="o", bufs=3)
                oB = psp.tile([128, 512], f32, tag="o", bufs=3)
                nkb = 4 * (qb + 1)
                for kb in range(nkb):
                    kt_t = ktr[kb // 4]
                    kcols = slice((kb % 4) * 128, (kb % 4) * 128 + 128)
                    jrel = kb - 4 * qb
                    lo = max(jrel, 0) * 128   # first valid q col in block
                    sub = slice(lo, 512)
                    stA = psp.tile([128, 512], f32, tag="st", bufs=3)
                    stB = psp.tile([128, 512], f32, tag="st", bufs=3)
                    nc.tensor.matmul(stA[:, sub], kt_t[0:64, kcols],
                                     qcols_t[0:64, sub])
                    nc.tensor.matmul(stB[:, sub], kt_t[64:128, kcols],
                                     qcols_t[64:128, sub])
                    pA = til([128, 512], bf16, "p", 8)
                    pB = til([128, 512], bf16, "p", 8)
                    nc.scalar.activation(pA[:, sub], stA[:, sub], AF.Exp,
                                         scale=SCALE)
                    nc.scalar.activation(pB[:, sub], stB[:, sub], AF.Exp,
                                         scale=SCALE)
                    if jrel >= 0:
                        tri = slice(lo, lo + 128)
                        nc.vector.tensor_mul(pA[:, tri], pA[:, tri], mask_t[:])
                        nc.vector.tensor_mul(pB[:, tri], pB[:, tri], mask_t[:])
                    nc.tensor.matmul(oA[:, sub], v_slice(kb, p, 0, 128),
                                     pA[:, sub],
                                     start=(kb == 0), stop=(kb == nkb - 1))
                    nc.tensor.matmul(oB[:, sub], v_slice(kb, p, 64, 192),
                                     pB[:, sub],
                                     start=(kb == 0), stop=(kb == nkb - 1))
                # normalize. A psum rows: [O_A | l_A]; B psum rows: [l_B | O_B]
                qcols = slice(qb * 512, (qb + 1) * 512)
                onrm = til([128, 512], bf16, "onrm", 4)
                rcA = til([128, 512], f32, "rcA", 2)
                nc.vector.reciprocal(rcA[64:128, :], oA[64:128, :])
                rcA2 = til([64, 512], f32, "rcA2", 3)
                nc.sync.dma_start(rcA2[:], rcA[64:128, :])
                nc.vector.tensor_mul(onrm[0:64, :], oA[0:64, :], rcA2[:])
                rcB = til([64, 512], f32, "rcB", 3)
                nc.vector.reciprocal(rcB[:], oB[0:64, :])
                rcB2 = til([128, 512], f32, "rcB2", 2)
                nc.sync.dma_start(rcB2[64:128, :], rcB[:])
                nc.vector.tensor_mul(onrm[64:128, :], oB[64:128, :],
                                     rcB2[64:128, :])
                nc.sync.dma_start(og_send[p][:, qcols].opt(), onrm[:])

        # ------------- per-pair exchange + output projection ---------------
        ofull = [None] * 8
        for p in range(4):
            if timing:
                nc.sync.dma_start(og_recv[p][0:128, :].opt(),
                                  og_send[p][:].opt())
                nc.sync.dma_start(og_recv[p][128:256, :].opt(),
                                  og_send[p][:].opt())
            else:
                nc.gpsimd.collective_compute(
                    "AllGather", mybir.AluOpType.bypass, replica_groups=GROUPS,
                    ins=[og_send[p][:].opt()], outs=[og_recv[p][:].opt()],
                )
            for g2 in range(2):
                t = til([128, S], bf16, "of" if p < 3 else "xt",
                        6 if p < 3 else 8)
                nc.sync.dma_start(
                    t[:], og_recv[p][g2 * 128:(g2 + 1) * 128, :].opt())
                ofull[4 * g2 + p] = t
        wt = load_w(woT)
        # two-pass accumulation: pass A (pairs 0-1) runs during the pair-2/3
        # attention, parking partials in DRAM (one tensor per st16 so the
        # passes pipeline); pass B adds pairs 2-3
        for st16 in range(16):
            ps = psp.tile([128, 512], f32, tag="proj", bufs=2)
            for i, dt in enumerate([0, 4, 1, 5]):
                nc.tensor.matmul(
                    ps[:],
                    ofull[dt][:, st16 * 128:(st16 + 1) * 128],
                    wt[dt][:],
                    start=(i == 0), stop=(i == 3),
                )
            a_sb = til([128, SL], f32, "osb", 4)
            nc.vector.tensor_copy(a_sb[:], ps[:])
            nc.sync.dma_start(acc_d[st16][:].opt(), a_sb[:])
        for st16 in range(16):
            a_rd = til([128, SL], f32, "ard", 8)
            nc.sync.dma_start(a_rd[:], acc_d[st16][:].opt())
            ps = psp.tile([128, 512], f32, tag="proj", bufs=2)
            for i, dt in enumerate([2, 6, 3, 7]):
                nc.tensor.matmul(
                    ps[:],
                    ofull[dt][:, st16 * 128:(st16 + 1) * 128],
                    wt[dt][:],
                    start=(i == 0), stop=(i == 3),
                )
            o_sb = til([128, SL], f32, "osb", 4)
            nc.vector.tensor_add(o_sb[:], ps[:], a_rd[:])
            nc.sync.dma_start(out[st16 * 128:(st16 + 1) * 128, :], o_sb[:])

def rope_perm_rows(heads):
    rows = []
    for h in heads:
        rows += [h * DH + j for j in range(0, DH, 2)]
        rows += [h * DH + j for j in range(1, DH, 2)]
    return np.array(rows)


def prep_inputs(x, WQ, WK, WV, WO, token_positions):
    x = np.asarray(x, np.float32)
    WQ = np.asarray(WQ, np.float32)
    WK = np.asarray(WK, np.float32)
    WV = np.asarray(WV, np.float32)
    WO = np.asarray(WO, np.float32)
    pos = np.asarray(token_positions).astype(np.float32)
    bf = ml_dtypes.bfloat16

    r = np.arange(128)
    invf = (10000.0 ** (-(r % 32) / 32.0)).astype(np.float32)
    sign = np.where((r % 64) < 32, -1.0, 1.0).astype(np.float32)
    ang = pos[None, :] * invf[:, None]
    cosr = np.cos(ang).astype(np.float32)
    sinr = np.sin(ang * sign[:, None]).astype(np.float32)

    in_maps = []
    for c in range(NCORE):
        b, g = divmod(c, 2)
        heads = list(range(8 * g, 8 * g + 8))
        perm = rope_perm_rows(heads)
        rows = slice(8 * g * DH, (8 * g + 8) * DH)
        in_maps.append({
            "xT": np.ascontiguousarray(x[b].T).astype(bf),
            "wqT": np.ascontiguousarray(WQ[perm, :].T).astype(bf),
            "wkT": np.ascontiguousarray(WK[perm, :].T).astype(bf),
            "wvT": np.ascontiguousarray(WV.T[:, rows]).astype(bf),
            "woT": np.ascontiguousarray(WO.T[:, g * SL:(g + 1) * SL]).astype(bf),
            "cosr": cosr,
            "sinr": sinr,
        })
    return in_maps


def assemble(results):
    B = NCORE // 2
    out = np.empty((B, S, D), np.float32)
    for b in range(B):
        out[b, :, 0:SL] = results[2 * b]["out"]
        out[b, :, SL:D] = results[2 * b + 1]["out"]
    return out


_NC = None


def _get_nc():
    global _NC
    if _NC is None:
        _NC = build()
    return _NC


def kernel(x, WQ, WK, WV, WO, token_positions):
    nc = _get_nc()
    in_maps = prep_inputs(x, WQ, WK, WV, WO, token_positions)
    res = run_bass_kernel_spmd(nc, in_maps, list(range(NCORE)))
    return assemble(res.results)



# revision 36
# speedup vs baseline: 1.1010x; 1.1010x over previous
"""Trainium2 Bass kernel: multi-head flash self-attention with RoPE.

Problem: x[4,2048,1024], 16 heads, dh=64, causal, RoPE(theta=10000), WO proj.

Sharding (8 cores): core c -> batch b=c//2, head-group g=c%2 (8 heads each).

v2 design notes:
  - Q/K/V projections in fp8e4m3 DoubleRow matmuls with a 3-term hi/lo
    split (x_hi*w_hi + x_lo*w_hi + x_hi*w_lo): bf16-level accuracy at
    0.75x the bf16 PE cost. Operands host-prepped in the DoubleRow
    plane-paired layout [d_part, 2, cols] (planes = D-slabs j, j+4),
    scaled by 2^5 (x) and 2^7 (w); the 2^-12 undo is folded into the
    RoPE tables (Q/K) and the V-evacuation copy.
  - RoPE row layout puts each head's pair-halves in 16-row blocks so the
    rotate-half swap is a DVE stream_shuffle (quadrant-local), no DMAs.
    Muls/adds split across DVE and GpSimd.
  - Flash attention in S^T layout ([k,q] blocks), heads A/B fused: scores
    for both heads land in one 2-bank PSUM tile [128,1024]; ONE scalar-
    engine exp per k-block covers both heads (3D access pattern). V is
    stored per (ktile, head-pair) as [V_A | ones | V_B] so softmax
    denominators come out of the PV matmul for free.
  - Per-pair AllGather of normalized O^T (pair 3 exchanged per-qb to
    shorten the tail); output projection accumulates per-pair-group into
    SBUF (passes A/B/C), final store in bf16.
"""
import sys

sys.path.insert(0, "/opt/trn_rl_repo")

import numpy as np
import ml_dtypes
import concourse.bass as bass
import concourse.bacc as bacc
import concourse.mybir as mybir
from concourse import tile
from concourse.bass_utils import run_bass_kernel_spmd

f32 = mybir.dt.float32
bf16 = mybir.dt.bfloat16
fp8 = mybir.dt.float8e4
AF = mybir.ActivationFunctionType
DR = mybir.MatmulPerfMode.DoubleRow
E4 = ml_dtypes.float8_e4m3

S = 2048
D = 1024
H = 16
DH = 64
NCORE = 8
SL = 512           # local m dims (8 heads x 64)
SCALE = 1.0 / 8.0  # 1/sqrt(dh)
GROUPS = [[0, 1], [2, 3], [4, 5], [6, 7]]
VPP = 192          # v_store cols per (ktile, pair): [V_A | ones | V_B]
VKT = 4 * VPP      # v_store cols per ktile
XSH = 5            # x quant scale 2^5
WSH = 7            # w quant scale 2^7
UNDO = 2.0 ** (-(XSH + WSH))
SWAP_MASK = list(range(16, 32)) + list(range(16))  # rotate-half swap


def build(timing=False):
    nc = bacc.Bacc("TRN2", target_bir_lowering=False, debug=False,
                   num_devices=1 if timing else NCORE)

    x8 = {}
    for hl in "hl":
        for j in range(4):
            x8[hl, j] = nc.dram_tensor(f"x8{hl}{j}", [128, 2 * S], fp8,
                                       kind="ExternalInput").ap()
    w8 = {}
    for w in ("wq", "wk", "wv"):
        for hl in "hl":
            for j in range(4):
                w8[w, hl, j] = nc.dram_tensor(f"{w}8{hl}{j}", [128, 1024],
                                              fp8, kind="ExternalInput").ap()
    woT = nc.dram_tensor("woT", [D, SL], bf16, kind="ExternalInput").ap()
    cosr = nc.dram_tensor("cosr", [128, S], f32, kind="ExternalInput").ap()
    sinr = nc.dram_tensor("sinr", [128, S], f32, kind="ExternalInput").ap()
    out = nc.dram_tensor("out", [S, SL], bf16, kind="ExternalOutput").ap()

    og_send = [nc.dram_tensor(f"og_send{p}", [128, S], bf16) for p in range(3)]
    og_recv = [nc.dram_tensor(f"og_recv{p}", [256, S], bf16) for p in range(3)]
    # pair 3 exchanges per-qb; collectives need contiguous dram patterns
    og_send.append([nc.dram_tensor(f"og_send3_{qb}", [128, 512], bf16)
                    for qb in range(4)])
    og_recv.append([nc.dram_tensor(f"og_recv3_{qb}", [256, 512], bf16)
                    for qb in range(4)])

    with tile.TileContext(nc) as tc:
        _body(nc, tc, x8, w8, woT, cosr, sinr, out, og_send, og_recv, timing)
    nc.compile()
    return nc


def _body(nc, tc, x8, w8, woT, cosr, sinr, out, og_send, og_recv,
          timing=False):
    from contextlib import ExitStack
    ctx = ExitStack()
    with ctx:
        sb = ctx.enter_context(tc.tile_pool(name="sb", bufs=1))
        psp = ctx.enter_context(tc.tile_pool(name="psp", bufs=1, space="PSUM"))
        counter = [0]

        def til(shape, dtype, tag, bufs):
            counter[0] += 1
            return sb.tile(shape, dtype, tag=tag, bufs=bufs,
                           name=f"{tag}_{counter[0]}")

        # ---------------- input loads, consumption order ----------------
        # phase 1: x cols 0:512 (both planes) + V weights -> first V quarter
        xt = {}     # (hl, j) -> [128, 4096] fp8 tile, cols = plane*2048 + s
        wvt = {}
        wqt = {}
        wkt = {}

        def x_3d(hl, j):
            return xt[hl, j][:].rearrange("p (j s) -> p j s", j=2)

        # "l"-plane inputs load via the gpsimd SWDGE queue, "h" via SP's
        # HWDGE — halves the serialized per-DMA overhead at startup.
        def eng(hl):
            return nc.sync

        def load_x_phase(ph):
            cols = slice(ph * 512, (ph + 1) * 512)
            for hl in "hl":
                for j in range(4):
                    eng(hl).dma_start(
                        x_3d(hl, j)[:, :, cols],
                        x8[hl, j][:].rearrange("p (j s) -> p j s", j=2)
                        [:, :, cols])

        # gpsimd constants first so they don't queue behind SWDGE loads
        # PE-side causal mask: scores PSUM gets += U^T @ (-BIG*I) on the
        # diagonal 128x128 sub-block, i.e. -1e30 where q < k, so the exp
        # yields exact zeros with no post-exp mask op.
        # U[k, r] = 1 where k < r  (strict lower triangle as lhsT)
        u_t = til([128, 128], bf16, "um", 1)
        nc.gpsimd.memset(u_t[:], 1.0)
        nc.gpsimd.affine_select(
            out=u_t[:], in_=u_t[:], compare_op=mybir.AluOpType.is_gt,
            fill=0.0, base=0, pattern=[[1, 128]], channel_multiplier=-1,
        )
        # IBIG = diag(-1e30)
        ibig_t = til([128, 128], bf16, "ibig", 1)
        nc.gpsimd.memset(ibig_t[:], -1e30)
        nc.gpsimd.affine_select(
            out=ibig_t[:], in_=ibig_t[:], compare_op=mybir.AluOpType.is_ge,
            fill=0.0, base=0, pattern=[[1, 128]], channel_multiplier=-1,
        )
        nc.gpsimd.affine_select(
            out=ibig_t[:], in_=ibig_t[:], compare_op=mybir.AluOpType.is_ge,
            fill=0.0, base=0, pattern=[[-1, 128]], channel_multiplier=1,
        )
        # v quarters: v_q[i] holds ktiles 4i..4i+4; per (kt, pair p) block
        # of VPP cols: [V_A | ones | V_B]
        v_q = []
        for i in range(16):
            vq = til([128, VKT], bf16, "v", 16)
            nc.gpsimd.memset(vq[:], 1.0)
            v_q.append(vq)

        for hl in "hl":
            for j in range(4):
                wvt[hl, j] = til([128, 1024], fp8, "wv", 8)
                xt[hl, j] = til([128, 2 * S], fp8, "x8", 8)
        for j in range(4):
            for hl in "hl":
                eng(hl).dma_start(wvt[hl, j][:], w8["wv", hl, j][:])
                eng(hl).dma_start(
                    x_3d(hl, j)[:, :, 0:512],
                    x8[hl, j][:].rearrange("p (j s) -> p j s", j=2)
                    [:, :, 0:512])
        for j in range(4):
            for hl in "hl":
                wqt[hl, j] = til([128, 1024], fp8, "wq", 8)
                eng(hl).dma_start(wqt[hl, j][:], w8["wq", hl, j][:])
                wkt[hl, j] = til([128, 1024], fp8, "wk", 8)
                eng(hl).dma_start(wkt[hl, j][:], w8["wk", hl, j][:])
        cos_t = til([128, S], f32, "cos", 1)
        nc.sync.dma_start(cos_t[:, 0:512], cosr[:, 0:512])
        sin_t = til([128, S], f32, "sin", 1)
        nc.gpsimd.dma_start(sin_t[:, 0:512], sinr[:, 0:512])
        load_x_phase(1)
        nc.sync.dma_start(cos_t[:, 512:2048], cosr[:, 512:2048])
        nc.gpsimd.dma_start(sin_t[:, 512:2048], sinr[:, 512:2048])
        load_x_phase(2)
        load_x_phase(3)
        wt = []
        for dt in range(8):
            t = til([128, SL], bf16, "wo", 8)
            nc.sync.dma_start(t[:], woT[dt * 128:(dt + 1) * 128, :])
            wt.append(t)

        def proj_mms(ps, stat_of, mov_of):
            """12 DoubleRow matmuls, term-major so they track DMA arrival."""
            terms = [("h", "h"), ("l", "h"), ("h", "l")]
            n = 0
            for (a, b) in terms:
                for j in range(4):
                    n += 1
                    nc.tensor.matmul(
                        ps, stat_of(a, j), mov_of(b, j),
                        start=(n == 1), stop=(n == 12),
                        perf_mode=DR,
                    )

        def emit_v_quarter(i):
            for kt4 in range(4):
                kt = 4 * i + kt4
                ps = psp.tile([128, 512], f32, tag="ps5", bufs=4)
                proj_mms(
                    ps[:],
                    lambda a, j: x_3d(a, j)[:, :, kt * 128:(kt + 1) * 128],
                    lambda b, j: wvt[b, j][:].rearrange(
                        "p (j m) -> p j m", j=2),
                )
                vva = v_q[kt][:].rearrange("q (a c) -> q a c", c=64)
                psa = ps[:].rearrange("q (a c) -> q a c", c=64)
                nc.vector.tensor_scalar_mul(
                    vva[:, 0:12:3, :], psa[:, 0:8:2, :], UNDO)
                nc.vector.tensor_scalar_mul(
                    vva[:, 2:12:3, :], psa[:, 1:8:2, :], UNDO)

        def v_slice(kt, p, c0, c1):
            off = p * VPP
            return v_q[kt][:, off + c0:off + c1]

        from collections import deque
        filler = deque()   # single-MM thunks of attention-independent work

        def pump(n):
            k = 0
            while filler and k < n:
                filler.popleft()()
                k += 1

        def flush_filler():
            while filler:
                filler.popleft()()

        # per-st projection + rope into a [128, 512] bf16 tile. When
        # eager=False the 12 matmuls are enqueued as filler thunks; the
        # rope chain is emitted by the last thunk.
        def proj_rope_st(wtiles, mt, st, eager=True):
            big_t = til([128, 512], bf16, "qk", 17)
            ps = psp.tile([128, 512], f32, tag="ps5", bufs=4)

            def stat(a, j):
                return wtiles[a, j][:].rearrange(
                    "p (j m) -> p j m", j=2)[:, :, mt * 128:(mt + 1) * 128]

            def mov(b, j):
                return x_3d(b, j)[:, :, st * 512:(st + 1) * 512]

            def rope():
                # prefix (eager) runs the whole chain on DVE (Pool busy
                # with SWDGE input loads then); filler ropes split DVE/Pool
                cols = slice(st * 512, (st + 1) * 512)
                tmp = til([128, 512], f32, "tmp", 2)
                nc.vector.tensor_mul(tmp[:], ps[:], cos_t[:, cols])
                swp = til([128, 512], f32, "swp", 2)
                nc.vector.stream_shuffle(swp[:], ps[:], SWAP_MASK)
                swp2 = til([128, 512], f32, "swp2", 2)
                mulv = nc.vector if eager else nc.gpsimd
                mulv.tensor_mul(swp2[:], swp[:], sin_t[:, cols])
                mulv.tensor_add(big_t[:], tmp[:], swp2[:])

            terms = [("h", "h"), ("l", "h"), ("h", "l")]
            steps = [(n, a, b, j) for n, (a, b, j) in enumerate(
                (a, b, j) for (a, b) in terms for j in range(4))]

            def mk(n, a, b, j):
                def thunk():
                    nc.tensor.matmul(ps[:], stat(a, j), mov(b, j),
                                     start=(n == 0), stop=(n == 11),
                                     perf_mode=DR)
                    if n == 11:
                        rope()
                return thunk

            for (n, a, b, j) in steps:
                t = mk(n, a, b, j)
                if eager:
                    t()
                else:
                    filler.append(t)
            return big_t

        ofull = [[None, None] for _ in range(4)]  # [pair][member]

        def exchange_pair(p, qb=None):
            """AllGather pair p's O^T (whole pair, or one qb slice)."""
            if qb is None:
                qcols = slice(0, S)
                snd, rcv = og_send[p][:], og_recv[p]
            else:
                qcols = slice(qb * 512, (qb + 1) * 512)
                snd, rcv = og_send[3][qb][:], og_recv[3][qb]
            if timing:
                # stub the AllGather as two gpsimd-queue (SWDGE) copies,
                # mirroring the real collective's Pool-engine placement
                nc.gpsimd.dma_start(rcv[0:128, :].opt(), snd.opt())
                nc.gpsimd.dma_start(rcv[128:256, :].opt(), snd.opt())
            else:
                nc.gpsimd.collective_compute(
                    "AllGather", mybir.AluOpType.bypass,
                    replica_groups=GROUPS,
                    ins=[snd.opt()], outs=[rcv[:].opt()],
                )
            for g2 in range(2):
                if ofull[p][g2] is None:
                    ofull[p][g2] = til([128, S], bf16, "of", 6)
                nc.sync.dma_start(
                    ofull[p][g2][:, qcols],
                    rcv[g2 * 128:(g2 + 1) * 128, :].opt())

        # -------- per pair: Q/K projection + rope + flash attention --------
        acc = [None] * 16   # SBUF accumulators for the output projection

        def attention_qb(p, qb, qtr, ktr):
            qcols_t = qtr[qb]
            oA = psp.tile([128, 512], f32, tag="ps5", bufs=4)
            oB = psp.tile([128, 512], f32, tag="ps5", bufs=4)
            nkb = 4 * (qb + 1)

            def emit_scores(kb):
                kt_t = ktr[kb // 4]
                kcols = slice((kb % 4) * 128, (kb % 4) * 128 + 128)
                jrel = kb - 4 * qb
                lo = max(jrel, 0) * 128   # first valid q col in block
                sub = slice(lo, 512)
                stAB = psp.tile([128, 1024], f32, tag="st", bufs=2)
                diag = jrel >= 0
                nc.tensor.matmul(stAB[:, lo:512], kt_t[0:64, kcols],
                                 qcols_t[0:64, sub],
                                 start=True, stop=not diag)
                nc.tensor.matmul(stAB[:, 512 + lo:1024],
                                 kt_t[64:128, kcols],
                                 qcols_t[64:128, sub],
                                 start=True, stop=not diag)
                if diag:
                    # accumulate -1e30 on the q<k triangle of the 128-wide
                    # diagonal sub-block (both heads) via the PE
                    nc.tensor.matmul(stAB[:, lo:lo + 128], u_t[:],
                                     ibig_t[:], start=False, stop=True)
                    nc.tensor.matmul(stAB[:, 512 + lo:512 + lo + 128],
                                     u_t[:], ibig_t[:],
                                     start=False, stop=True)
                pAB = til([128, 1024], bf16, "p", 6)
                st3 = stAB[:].rearrange("p (j c) -> p j c", j=2)
                p3 = pAB[:].rearrange("p (j c) -> p j c", j=2)
                nc.scalar.activation(p3[:, :, sub], st3[:, :, sub],
                                     AF.Exp, scale=SCALE)
                return pAB, lo, sub

            def emit_pv(kb, pAB, lo, sub):
                nc.tensor.matmul(oA[:, sub], v_slice(kb, p, 0, 128),
                                 pAB[:, sub],
                                 start=(kb == 0), stop=(kb == nkb - 1))
                nc.tensor.matmul(oB[:, sub], v_slice(kb, p, 64, 192),
                                 pAB[:, 512 + lo:1024],
                                 start=(kb == 0), stop=(kb == nkb - 1))

            # one-block software pipeline: PV(kb) is emitted after
            # scores(kb+1) plus a few filler matmuls, so the exp latency
            # hides behind PE work
            rate = 2 if qb < 3 else 3
            prev = None
            for kb in range(nkb):
                cur = emit_scores(kb)
                pump(rate)
                if prev is not None:
                    emit_pv(kb - 1, *prev)
                prev = cur
            pump(rate)
            emit_pv(nkb - 1, *prev)
            # normalize. A psum rows: [O_A | l_A]; B psum rows: [l_B | O_B]
            qcols = slice(qb * 512, (qb + 1) * 512)
            onrm = til([128, 512], bf16, "onrm", 4)
            rc = til([128, 512], f32, "rc", 2)
            nc.vector.reciprocal(rc[64:128, :], oA[64:128, :])
            nc.vector.reciprocal(rc[0:64, :], oB[0:64, :])
            rc2 = til([128, 512], f32, "rc2", 2)
            nc.sync.dma_start(rc2[0:64, :], rc[64:128, :])
            nc.sync.dma_start(rc2[64:128, :], rc[0:64, :])
            nc.vector.tensor_mul(onrm[0:64, :], oA[0:64, :], rc2[0:64, :])
            nc.vector.tensor_mul(onrm[64:128, :], oB[64:128, :],
                                 rc2[64:128, :])
            if p == 3:
                nc.sync.dma_start(og_send[3][qb][:].opt(), onrm[:])
                exchange_pair(3, qb)
            else:
                nc.sync.dma_start(og_send[p][:, qcols].opt(), onrm[:])

        # output projection passes: A = pairs {0,1}, B = pair 2, C = pair 3
        def out_group(dts, kind, st16, ofin=None):
            cols = slice(st16 * 128, (st16 + 1) * 128)
            ps = psp.tile([128, 512], f32, tag="ps5", bufs=4)

            def tail():
                if kind == "A":
                    acc[st16] = til([128, SL], bf16, "osb", 16)
                    nc.vector.tensor_copy(acc[st16][:], ps[:])
                elif kind == "B":
                    nc.vector.tensor_add(acc[st16][:], ps[:], acc[st16][:])
                else:
                    nc.vector.tensor_add(
                        ofin[:, (st16 % 4) * 512:(st16 % 4 + 1) * 512],
                        ps[:], acc[st16][:])

            thunks = []
            for i, (p, g2) in enumerate(dts):
                def mk(i, p, g2):
                    def thunk():
                        nc.tensor.matmul(
                            ps[:], ofull[p][g2][:, cols], wt[4 * g2 + p][:],
                            start=(i == 0), stop=(i == len(dts) - 1),
                        )
                        if i == len(dts) - 1:
                            tail()
                    return thunk
                thunks.append(mk(i, p, g2))
            return thunks

        def out_pass(dts, kind, st16s, eager=True):
            for st16 in st16s:
                for t in out_group(dts, kind, st16):
                    if eager:
                        t()
                    else:
                        filler.append(t)

        # ---------------- schedule ----------------
        # p0 prefix: V quarters interleaved with p0's Q/K proj+rope
        trq = {0: [None] * 4}
        trk = {0: [None] * 4}
        for st in range(4):
            emit_v_quarter(st)
            trq[0][st] = proj_rope_st(wqt, 0, st)
            trk[0][st] = proj_rope_st(wkt, 0, st)

        for p in range(4):
            if p > 0:
                exchange_pair(p - 1)
            if p < 3:
                # next pair's Q/K projections as attention filler,
                # interleaved q0,k0,q1,k1,... so early tiles finish first
                trq[p + 1] = [None] * 4
                trk[p + 1] = [None] * 4
                for st in range(4):
                    trq[p + 1][st] = proj_rope_st(wqt, p + 1, st,
                                                  eager=False)
                    trk[p + 1][st] = proj_rope_st(wkt, p + 1, st,
                                                  eager=False)
            else:
                # output-projection passes A and B as filler during pair 3
                out_pass([(0, 0), (0, 1), (1, 0), (1, 1)], "A",
                         range(16), eager=False)
                out_pass([(2, 0), (2, 1)], "B", range(16), eager=False)
            for qb in range(4):
                attention_qb(p, qb, trq[p], trk[p])
            flush_filler()
        for qb in range(4):
            ofin = til([128, 4 * 512], bf16, "ofin", 2)
            for st16 in range(4 * qb, 4 * qb + 4):
                for t in out_group([(3, 0), (3, 1)], "C", st16, ofin=ofin):
                    t()
            nc.sync.dma_start(
                out[qb * 512:(qb + 1) * 512, :].rearrange(
                    "(a r) c -> r a c", a=4),
                ofin[:].rearrange("p (a c) -> p a c", a=4),
            )


def _rope_maps():
    """Partition layout within a head-pair tile row block.

    Per head (64 rows): [t1 of pairs 0:16 | t2 of pairs 0:16 |
                         t1 of pairs 16:32 | t2 of pairs 16:32]
    so the rotate-half swap exchanges 16-row blocks within each
    32-partition quadrant (stream_shuffle-expressible).

    Returns (j_idx[128], is_t2[128]) for one 128-row pair tile.
    """
    j_idx = np.zeros(128, np.int64)
    is_t2 = np.zeros(128, bool)
    for p in range(128):
        r = p % 32
        q2 = (p % 64) // 32
        j_idx[p] = q2 * 16 + (r % 16)
        is_t2[p] = r >= 16
    return j_idx, is_t2


def _qk_perm(heads):
    """W-row permutation for one core's 4 pair-tiles (512 rows)."""
    j_idx, is_t2 = _rope_maps()
    rows = []
    for mt in range(4):
        for p in range(128):
            h = heads[2 * mt + p // 64]
            dim = 2 * j_idx[p] + (1 if is_t2[p] else 0)
            rows.append(h * DH + dim)
    return np.array(rows)


def _quant_hi_lo(a, shift):
    s = float(2.0 ** shift)
    hi = np.clip(a * s, -224.0, 224.0).astype(E4)
    lo = (a * s - hi.astype(np.float32)).astype(E4)
    return hi, lo


def prep_inputs(x, WQ, WK, WV, WO, token_positions):
    x = np.asarray(x, np.float32)
    WQ = np.asarray(WQ, np.float32)
    WK = np.asarray(WK, np.float32)
    WV = np.asarray(WV, np.float32)
    WO = np.asarray(WO, np.float32)
    pos = np.asarray(token_positions).astype(np.float32)
    bf = ml_dtypes.bfloat16

    j_idx, is_t2 = _rope_maps()
    invf = (10000.0 ** (-j_idx.astype(np.float32) / 32.0))
    sign = np.where(is_t2, 1.0, -1.0).astype(np.float32)
    ang = pos[None, :] * invf[:, None]
    cosr = (np.cos(ang) * UNDO).astype(np.float32)
    sinr = (np.sin(ang * sign[:, None]) * UNDO).astype(np.float32)

    in_maps = []
    for c in range(NCORE):
        b, g = divmod(c, 2)
        heads = list(range(8 * g, 8 * g + 8))
        perm = _qk_perm(heads)
        rows = slice(8 * g * DH, (8 * g + 8) * DH)

        m = {"cosr": cosr, "sinr": sinr,
             "woT": np.ascontiguousarray(
                 WO.T[:, g * SL:(g + 1) * SL]).astype(bf)}

        # x planes: x8{h,l}{j}[p, plane*2048 + s] = q(x[b, s, (j+4*plane)*128+p])
        xT = x[b].T  # [D, S]
        xh, xl = _quant_hi_lo(xT, XSH)
        for j in range(4):
            m[f"x8h{j}"] = np.ascontiguousarray(
                np.concatenate([xh[j * 128:(j + 1) * 128, :],
                                xh[(j + 4) * 128:(j + 5) * 128, :]], axis=1))
            m[f"x8l{j}"] = np.ascontiguousarray(
                np.concatenate([xl[j * 128:(j + 1) * 128, :],
                                xl[(j + 4) * 128:(j + 5) * 128, :]], axis=1))

        # wq/wk: permuted rows -> [d, m] = W_perm.T; planes along d
        for name, W in (("wq", WQ), ("wk", WK)):
            WpT = np.ascontiguousarray(W[perm, :].T)  # [D, 512]
            wh, wl = _quant_hi_lo(WpT, WSH)
            for j in range(4):
                m[f"{name}8h{j}"] = np.ascontiguousarray(np.concatenate(
                    [wh[j * 128:(j + 1) * 128, :],
                     wh[(j + 4) * 128:(j + 5) * 128, :]], axis=1))
                m[f"{name}8l{j}"] = np.ascontiguousarray(np.concatenate(
                    [wl[j * 128:(j + 1) * 128, :],
                     wl[(j + 4) * 128:(j + 5) * 128, :]], axis=1))

        # wv: [d, m] = WV.T[:, this core's head rows]; planes along d
        WvT = np.ascontiguousarray(WV.T[:, rows])  # [D, 512]
        wh, wl = _quant_hi_lo(WvT, WSH)
        for j in range(4):
            m[f"wv8h{j}"] = np.ascontiguousarray(np.concatenate(
                [wh[j * 128:(j + 1) * 128, :],
                 wh[(j + 4) * 128:(j + 5) * 128, :]], axis=1))
            m[f"wv8l{j}"] = np.ascontiguousarray(np.concatenate(
                [wl[j * 128:(j + 1) * 128, :],
                 wl[(j + 4) * 128:(j + 5) * 128, :]], axis=1))

        in_maps.append(m)
    return in_maps


def assemble(results):
    B = NCORE // 2
    out = np.empty((B, S, D), np.float32)
    for b in range(B):
        out[b, :, 0:SL] = results[2 * b]["out"].astype(np.float32)
        out[b, :, SL:D] = results[2 * b + 1]["out"].astype(np.float32)
    return out


_NC = None


def _get_nc():
    global _NC
    if _NC is None:
        _NC = build()
    return _NC


def kernel(x, WQ, WK, WV, WO, token_positions):
    nc = _get_nc()
    in_maps = prep_inputs(x, WQ, WK, WV, WO, token_positions)
    res = run_bass_kernel_spmd(nc, in_maps, list(range(NCORE)))
    return assemble(res.results)


# revision 63
# speedup vs baseline: 1.1339x; 1.0299x over previous
"""Trainium2 Bass kernel: multi-head flash self-attention with RoPE.

Problem: x[4,2048,1024], 16 heads, dh=64, causal, RoPE(theta=10000), WO proj.

Sharding (8 cores): core c -> batch b=c//2, head-group g=c%2 (8 heads each).

v2 design notes:
  - Q/K/V projections in fp8e4m3 DoubleRow matmuls with a 3-term hi/lo
    split (x_hi*w_hi + x_lo*w_hi + x_hi*w_lo): bf16-level accuracy at
    0.75x the bf16 PE cost. Operands host-prepped in the DoubleRow
    plane-paired layout [d_part, 2, cols] (planes = D-slabs j, j+4),
    scaled by 2^5 (x) and 2^7 (w); the 2^-12 undo is folded into the
    RoPE tables (Q/K) and the V-evacuation copy.
  - RoPE row layout puts each head's pair-halves in 16-row blocks so the
    rotate-half swap is a DVE stream_shuffle (quadrant-local), no DMAs.
    Muls/adds split across DVE and GpSimd.
  - Flash attention in S^T layout ([k,q] blocks), heads A/B fused: scores
    for both heads land in one 2-bank PSUM tile [128,1024]; ONE scalar-
    engine exp per k-block covers both heads (3D access pattern). V is
    stored per (ktile, head-pair) as [V_A | ones | V_B] so softmax
    denominators come out of the PV matmul for free.
  - Per-pair AllGather of normalized O^T (pair 3 exchanged per-qb to
    shorten the tail); output projection accumulates per-pair-group into
    SBUF (passes A/B/C), final store in bf16.
"""
import sys

sys.path.insert(0, "/opt/trn_rl_repo")

import numpy as np
import ml_dtypes
import concourse.bass as bass
import concourse.bacc as bacc
import concourse.mybir as mybir
from concourse import tile
from concourse.bass_utils import run_bass_kernel_spmd

f32 = mybir.dt.float32
bf16 = mybir.dt.bfloat16
fp8 = mybir.dt.float8e4
AF = mybir.ActivationFunctionType
DR = mybir.MatmulPerfMode.DoubleRow
E4 = ml_dtypes.float8_e4m3

S = 2048
D = 1024
H = 16
DH = 64
NCORE = 8
SL = 512           # local m dims (8 heads x 64)
SCALE = 1.0 / 8.0  # 1/sqrt(dh)
GROUPS = [[0, 1], [2, 3], [4, 5], [6, 7]]
VPP = 192          # v_store cols per (ktile, pair): [V_A | ones | V_B]
VKT = 4 * VPP      # v_store cols per ktile
XSH = 5            # x quant scale 2^5
WSH = 7            # w quant scale 2^7
UNDO = 2.0 ** (-(XSH + WSH))
SWAP_MASK = list(range(16, 32)) + list(range(16))  # rotate-half swap


def build(timing=False):
    nc = bacc.Bacc("TRN2", target_bir_lowering=False, debug=False,
                   num_devices=1 if timing else NCORE)

    x8 = {}
    for hl in "hl":
        for j in range(4):
            x8[hl, j] = nc.dram_tensor(f"x8{hl}{j}", [128, 2 * S], fp8,
                                       kind="ExternalInput").ap()
    w8 = {}
    for w in ("wq", "wk", "wv"):
        for hl in "hl":
            for j in range(4):
                w8[w, hl, j] = nc.dram_tensor(f"{w}8{hl}{j}", [128, 1024],
                                              fp8, kind="ExternalInput").ap()
    woT = nc.dram_tensor("woT", [D, SL], bf16, kind="ExternalInput").ap()
    cosr = nc.dram_tensor("cosr", [128, S], f32, kind="ExternalInput").ap()
    sinr = nc.dram_tensor("sinr", [128, S], f32, kind="ExternalInput").ap()
    out = nc.dram_tensor("out", [S, SL], bf16, kind="ExternalOutput").ap()

    og_send = [nc.dram_tensor(f"og_send{p}", [128, S], bf16) for p in range(3)]
    og_recv = [nc.dram_tensor(f"og_recv{p}", [256, S], bf16) for p in range(3)]
    # pair 3 exchanges per-qb; collectives need contiguous dram patterns
    og_send.append([nc.dram_tensor(f"og_send3_{qb}", [128, 512], bf16)
                    for qb in range(4)])
    og_recv.append([nc.dram_tensor(f"og_recv3_{qb}", [256, 512], bf16)
                    for qb in range(4)])

    with tile.TileContext(nc) as tc:
        _body(nc, tc, x8, w8, woT, cosr, sinr, out, og_send, og_recv, timing)
    nc.compile()
    return nc


def _body(nc, tc, x8, w8, woT, cosr, sinr, out, og_send, og_recv,
          timing=False):
    from contextlib import ExitStack
    ctx = ExitStack()
    with ctx:
        sb = ctx.enter_context(tc.tile_pool(name="sb", bufs=1))
        psp = ctx.enter_context(tc.tile_pool(name="psp", bufs=1, space="PSUM"))
        counter = [0]

        def til(shape, dtype, tag, bufs):
            counter[0] += 1
            return sb.tile(shape, dtype, tag=tag, bufs=bufs,
                           name=f"{tag}_{counter[0]}")

        # ---------------- input loads, consumption order ----------------
        # phase 1: x cols 0:512 (both planes) + V weights -> first V quarter
        xt = {}     # (hl, j) -> [128, 4096] fp8 tile, cols = plane*2048 + s
        wvt = {}
        wqt = {}
        wkt = {}

        def x_3d(hl, j):
            return xt[hl, j][:].rearrange("p (j s) -> p j s", j=2)

        # "l"-plane inputs load via the gpsimd SWDGE queue, "h" via SP's
        # HWDGE — halves the serialized per-DMA overhead at startup.
        def eng(hl):
            return nc.sync

        def load_x_phase(ph):
            cols = slice(ph * 512, (ph + 1) * 512)
            for hl in "hl":
                for j in range(4):
                    eng(hl).dma_start(
                        x_3d(hl, j)[:, :, cols],
                        x8[hl, j][:].rearrange("p (j s) -> p j s", j=2)
                        [:, :, cols])

        # gpsimd constants first so they don't queue behind SWDGE loads
        # PE-side causal mask: scores PSUM gets += U^T @ (-BIG*I) on the
        # diagonal 128x128 sub-block, i.e. -1e30 where q < k, so the exp
        # yields exact zeros with no post-exp mask op.
        # U[k, r] = 1 where k < r  (strict lower triangle as lhsT)
        u_t = til([128, 128], bf16, "um", 1)
        nc.gpsimd.memset(u_t[:], 1.0)
        nc.gpsimd.affine_select(
            out=u_t[:], in_=u_t[:], compare_op=mybir.AluOpType.is_gt,
            fill=0.0, base=0, pattern=[[1, 128]], channel_multiplier=-1,
        )
        # IBIG = diag(-1e30)
        ibig_t = til([128, 128], bf16, "ibig", 1)
        nc.gpsimd.memset(ibig_t[:], -1e30)
        nc.gpsimd.affine_select(
            out=ibig_t[:], in_=ibig_t[:], compare_op=mybir.AluOpType.is_ge,
            fill=0.0, base=0, pattern=[[1, 128]], channel_multiplier=-1,
        )
        nc.gpsimd.affine_select(
            out=ibig_t[:], in_=ibig_t[:], compare_op=mybir.AluOpType.is_ge,
            fill=0.0, base=0, pattern=[[-1, 128]], channel_multiplier=1,
        )
        # v quarters: v_q[i] holds ktiles 4i..4i+4; per (kt, pair p) block
        # of VPP cols: [V_A | ones | V_B]
        v_q = []
        for i in range(16):
            vq = til([128, VKT], bf16, "v", 16)
            nc.gpsimd.memset(vq[:], 1.0)
            v_q.append(vq)

        for hl in "hl":
            for j in range(4):
                wvt[hl, j] = til([128, 1024], fp8, "wv", 8)
                xt[hl, j] = til([128, 2 * S], fp8, "x8", 8)
        for j in range(4):
            for hl in "hl":
                eng(hl).dma_start(wvt[hl, j][:], w8["wv", hl, j][:])
                eng(hl).dma_start(
                    x_3d(hl, j)[:, :, 0:512],
                    x8[hl, j][:].rearrange("p (j s) -> p j s", j=2)
                    [:, :, 0:512])
        for j in range(4):
            for hl in "hl":
                wqt[hl, j] = til([128, 1024], fp8, "wq", 8)
                eng(hl).dma_start(wqt[hl, j][:], w8["wq", hl, j][:])
                wkt[hl, j] = til([128, 1024], fp8, "wk", 8)
                eng(hl).dma_start(wkt[hl, j][:], w8["wk", hl, j][:])
        cos_t = til([128, S], f32, "cos", 1)
        nc.sync.dma_start(cos_t[:, 0:512], cosr[:, 0:512])
        sin_t = til([128, S], f32, "sin", 1)
        nc.gpsimd.dma_start(sin_t[:, 0:512], sinr[:, 0:512])
        load_x_phase(1)
        nc.sync.dma_start(cos_t[:, 512:2048], cosr[:, 512:2048])
        nc.gpsimd.dma_start(sin_t[:, 512:2048], sinr[:, 512:2048])
        load_x_phase(2)
        load_x_phase(3)
        wt = []
        for dt in range(8):
            t = til([128, SL], bf16, "wo", 8)
            nc.sync.dma_start(t[:], woT[dt * 128:(dt + 1) * 128, :])
            wt.append(t)

        def proj_mms(ps, stat_of, mov_of):
            """12 DoubleRow matmuls, term-major so they track DMA arrival."""
            terms = [("h", "h"), ("l", "h"), ("h", "l")]
            n = 0
            for (a, b) in terms:
                for j in range(4):
                    n += 1
                    nc.tensor.matmul(
                        ps, stat_of(a, j), mov_of(b, j),
                        start=(n == 1), stop=(n == 12),
                        perf_mode=DR,
                    )

        v_done = [False] * 16

        def emit_v_quarter(i, eager=True):
            for kt4 in range(4):
                kt = 4 * i + kt4
                cell = [None]

                def mk(kt, cell, n, a, b, j):
                    def thunk():
                        if n == 0:
                            counter[0] += 1
                            cell[0] = psp.tile([128, 512], f32,
                                               tag="ps5", bufs=4,
                                               name=f"vps_{counter[0]}")
                        nc.tensor.matmul(
                            cell[0][:],
                            x_3d(a, j)[:, :, kt * 128:(kt + 1) * 128],
                            wvt[b, j][:].rearrange("p (j m) -> p j m", j=2),
                            start=(n == 0), stop=(n == 11), perf_mode=DR)
                        if n == 11:
                            vva = v_q[kt][:].rearrange(
                                "q (a c) -> q a c", c=64)
                            psa = cell[0][:].rearrange(
                                "q (a c) -> q a c", c=64)
                            nc.vector.tensor_scalar_mul(
                                vva[:, 0:12:3, :], psa[:, 0:8:2, :], UNDO)
                            nc.vector.tensor_scalar_mul(
                                vva[:, 2:12:3, :], psa[:, 1:8:2, :], UNDO)
                            v_done[kt] = True
                    return thunk

                terms = [("h", "h"), ("l", "h"), ("h", "l")]
                for n, (a, b, j) in enumerate(
                        (a, b, j) for (a, b) in terms for j in range(4)):
                    t = mk(kt, cell, n, a, b, j)
                    if eager:
                        t()
                    else:
                        filler.append(t)
                if eager:
                    v_done[kt] = True

        def ensure_v(kt):
            while not v_done[kt]:
                filler.popleft()()

        def v_slice(kt, p, c0, c1):
            off = p * VPP
            return v_q[kt][:, off + c0:off + c1]

        from collections import deque
        filler = deque()   # single-MM thunks of attention-independent work

        def pump(n):
            k = 0
            while filler and k < n:
                filler.popleft()()
                k += 1

        def flush_filler():
            while filler:
                filler.popleft()()

        # per-st projection + rope into a [128, 512] bf16 tile. When
        # eager=False the 12 matmuls are enqueued as filler thunks; the
        # rope chain is emitted by the last thunk.
        def proj_rope_st(wtiles, mt, st, eager=True):
            big_t = til([128, 512], bf16, "qk", 17)
            ps = psp.tile([128, 512], f32, tag="ps5", bufs=4)

            def stat(a, j):
                return wtiles[a, j][:].rearrange(
                    "p (j m) -> p j m", j=2)[:, :, mt * 128:(mt + 1) * 128]

            def mov(b, j):
                return x_3d(b, j)[:, :, st * 512:(st + 1) * 512]

            def rope():
                # prefix (eager) runs the whole chain on DVE (Pool busy
                # with SWDGE input loads then); filler ropes split DVE/Pool
                cols = slice(st * 512, (st + 1) * 512)
                tmp = til([128, 512], f32, "tmp", 2)
                nc.vector.tensor_mul(tmp[:], ps[:], cos_t[:, cols])
                swp = til([128, 512], f32, "swp", 2)
                nc.vector.stream_shuffle(swp[:], ps[:], SWAP_MASK)
                swp2 = til([128, 512], f32, "swp2", 2)
                mulv = nc.vector if eager else nc.gpsimd
                mulv.tensor_mul(swp2[:], swp[:], sin_t[:, cols])
                mulv.tensor_add(big_t[:], tmp[:], swp2[:])

            terms = [("h", "h"), ("l", "h"), ("h", "l")]
            steps = [(n, a, b, j) for n, (a, b, j) in enumerate(
                (a, b, j) for (a, b) in terms for j in range(4))]

            def mk(n, a, b, j):
                def thunk():
                    nc.tensor.matmul(ps[:], stat(a, j), mov(b, j),
                                     start=(n == 0), stop=(n == 11),
                                     perf_mode=DR)
                    if n == 11:
                        rope()
                return thunk

            for (n, a, b, j) in steps:
                t = mk(n, a, b, j)
                if eager:
                    t()
                else:
                    filler.append(t)
            return big_t

        ofull = [[None, None] for _ in range(4)]  # [pair][member]

        def exchange_pair(p, qb=None):
            """AllGather pair p's O^T (whole pair, or one qb slice)."""
            if qb is None:
                qcols = slice(0, S)
                snd, rcv = og_send[p][:], og_recv[p]
            else:
                qcols = slice(qb * 512, (qb + 1) * 512)
                snd, rcv = og_send[3][qb][:], og_recv[3][qb]
            if timing:
                # stub the AllGather as two gpsimd-queue (SWDGE) copies,
                # mirroring the real collective's Pool-engine placement
                nc.gpsimd.dma_start(rcv[0:128, :].opt(), snd.opt())
                nc.gpsimd.dma_start(rcv[128:256, :].opt(), snd.opt())
            else:
                nc.gpsimd.collective_compute(
                    "AllGather", mybir.AluOpType.bypass,
                    replica_groups=GROUPS,
                    ins=[snd.opt()], outs=[rcv[:].opt()],
                )
            for g2 in range(2):
                if ofull[p][g2] is None:
                    ofull[p][g2] = til([128, S], bf16, "of", 6)
                nc.sync.dma_start(
                    ofull[p][g2][:, qcols],
                    rcv[g2 * 128:(g2 + 1) * 128, :].opt())

        # -------- per pair: Q/K projection + rope + flash attention --------
        acc = [None] * 16   # SBUF accumulators for the output projection

        def attention_qb(p, qb, qtr, ktr):
            qcols_t = qtr[qb]
            oA = psp.tile([128, 512], f32, tag="ps5", bufs=4)
            oB = psp.tile([128, 512], f32, tag="ps5", bufs=4)
            nkb = 4 * (qb + 1)

            def emit_scores(kb):
                kt_t = ktr[kb // 4]
                kcols = slice((kb % 4) * 128, (kb % 4) * 128 + 128)
                jrel = kb - 4 * qb
                lo = max(jrel, 0) * 128   # first valid q col in block
                sub = slice(lo, 512)
                stAB = psp.tile([128, 1024], f32, tag="st", bufs=2)
                diag = jrel >= 0
                nc.tensor.matmul(stAB[:, lo:512], kt_t[0:64, kcols],
                                 qcols_t[0:64, sub],
                                 start=True, stop=not diag)
                nc.tensor.matmul(stAB[:, 512 + lo:1024],
                                 kt_t[64:128, kcols],
                                 qcols_t[64:128, sub],
                                 start=True, stop=not diag)
                if diag:
                    # accumulate -1e30 on the q<k triangle of the 128-wide
                    # diagonal sub-block (both heads) via the PE
                    nc.tensor.matmul(stAB[:, lo:lo + 128], u_t[:],
                                     ibig_t[:], start=False, stop=True)
                    nc.tensor.matmul(stAB[:, 512 + lo:512 + lo + 128],
                                     u_t[:], ibig_t[:],
                                     start=False, stop=True)
                pAB = til([128, 1024], bf16, "p", 6)
                st3 = stAB[:].rearrange("p (j c) -> p j c", j=2)
                p3 = pAB[:].rearrange("p (j c) -> p j c", j=2)
                nc.scalar.activation(p3[:, :, sub], st3[:, :, sub],
                                     AF.Exp, scale=SCALE)
                return pAB, lo, sub

            def emit_pv(kb, pAB, lo, sub):
                nc.tensor.matmul(oA[:, sub], v_slice(kb, p, 0, 128),
                                 pAB[:, sub],
                                 start=(kb == 0), stop=(kb == nkb - 1))
                nc.tensor.matmul(oB[:, sub], v_slice(kb, p, 64, 192),
                                 pAB[:, 512 + lo:1024],
                                 start=(kb == 0), stop=(kb == nkb - 1))

            # one-block software pipeline: PV(kb) is emitted after
            # scores(kb+1) plus a few filler matmuls, so the exp latency
            # hides behind PE work
            # two-block software pipeline: PV(kb) trails scores(kb+2);
            # PSUM accumulation is order-independent so this is safe, and
            # stAB is freed by the exp, not the PV
            rate = 2 if qb < 3 else 3
            pend = []
            for kb in range(nkb):
                pend.append((kb,) + emit_scores(kb))
                pump(rate)
                if len(pend) > 2:
                    e = pend.pop(0)
                    if p == 0:
                        ensure_v(e[0])
                    emit_pv(*e)
            while pend:
                pump(rate)
                e = pend.pop(0)
                if p == 0:
                    ensure_v(e[0])
                emit_pv(*e)
            # normalize. A psum rows: [O_A | l_A]; B psum rows: [l_B | O_B]
            qcols = slice(qb * 512, (qb + 1) * 512)
            onrm = til([128, 512], bf16, "onrm", 4)
            rc = til([128, 512], f32, "rc", 2)
            nc.vector.reciprocal(rc[64:128, :], oA[64:128, :])
            nc.vector.reciprocal(rc[0:64, :], oB[0:64, :])
            rc2 = til([128, 512], f32, "rc2", 2)
            nc.sync.dma_start(rc2[0:64, :], rc[64:128, :])
            nc.sync.dma_start(rc2[64:128, :], rc[0:64, :])
            nc.vector.tensor_mul(onrm[0:64, :], oA[0:64, :], rc2[0:64, :])
            nc.vector.tensor_mul(onrm[64:128, :], oB[64:128, :],
                                 rc2[64:128, :])
            if p == 3:
                nc.sync.dma_start(og_send[3][qb][:].opt(), onrm[:])
                exchange_pair(3, qb)
            else:
                nc.sync.dma_start(og_send[p][:, qcols].opt(), onrm[:])

        # output projection passes: A = pairs {0,1}, B = pair 2, C = pair 3
        def out_group(dts, kind, st16, ofin=None):
            cols = slice(st16 * 128, (st16 + 1) * 128)
            cell = [None]

            def tail():
                ps = cell[0]
                if kind == "A":
                    acc[st16] = til([128, SL], bf16, "osb", 16)
                    nc.vector.tensor_copy(acc[st16][:], ps[:])
                elif kind == "B":
                    nc.vector.tensor_add(acc[st16][:], ps[:], acc[st16][:])
                else:
                    nc.vector.tensor_add(
                        ofin[:, (st16 % 4) * 512:(st16 % 4 + 1) * 512],
                        ps[:], acc[st16][:])

            thunks = []
            for i, (p, g2) in enumerate(dts):
                def mk(i, p, g2):
                    def thunk():
                        if i == 0:
                            counter[0] += 1
                            cell[0] = psp.tile([128, 512], f32,
                                               tag="ps5", bufs=4,
                                               name=f"ops_{counter[0]}")
                        nc.tensor.matmul(
                            cell[0][:], ofull[p][g2][:, cols],
                            wt[4 * g2 + p][:],
                            start=(i == 0), stop=(i == len(dts) - 1),
                        )
                        if i == len(dts) - 1:
                            tail()
                    return thunk
                thunks.append(mk(i, p, g2))
            return thunks

        def out_pass(dts, kind, st16s, eager=True):
            for st16 in st16s:
                for t in out_group(dts, kind, st16):
                    if eager:
                        t()
                    else:
                        filler.append(t)

        # ---------------- schedule ----------------
        # p0 prefix: V quarter 0 + p0's Q/K proj+rope eager; V quarters
        # 1-3 go to the filler queue (deadline-guarded by ensure_v)
        trq = {0: [None] * 4}
        trk = {0: [None] * 4}
        for st in range(4):
            emit_v_quarter(st)
            trq[0][st] = proj_rope_st(wqt, 0, st)
            trk[0][st] = proj_rope_st(wkt, 0, st)

        def emit_passC_qb(qb):
            ofin = til([128, 4 * 512], bf16, "ofin", 2)
            for half in range(2):
                for st16 in range(4 * qb + 2 * half, 4 * qb + 2 * half + 2):
                    for t in out_group([(3, 0), (3, 1)], "C", st16,
                                       ofin=ofin):
                        t()
                rows = slice(qb * 512 + half * 256, qb * 512 + half * 256
                             + 256)
                nc.sync.dma_start(
                    out[rows, :].rearrange("(a r) c -> r a c", a=2),
                    ofin[:, half * 1024:(half + 1) * 1024].rearrange(
                        "p (a c) -> p a c", a=2),
                )

        for p in range(4):
            if p > 0:
                exchange_pair(p - 1)
            if p < 3:
                # next pair's Q/K projections as attention filler,
                # interleaved q0,k0,q1,k1,... so early tiles finish first
                trq[p + 1] = [None] * 4
                trk[p + 1] = [None] * 4
                for st in range(4):
                    trq[p + 1][st] = proj_rope_st(wqt, p + 1, st,
                                                  eager=False)
                    trk[p + 1][st] = proj_rope_st(wkt, p + 1, st,
                                                  eager=False)
            else:
                # output-projection passes A and B as filler during pair 3
                out_pass([(0, 0), (0, 1), (1, 0), (1, 1)], "A",
                         range(16), eager=False)
                out_pass([(2, 0), (2, 1)], "B", range(16), eager=False)
            for qb in range(4):
                attention_qb(p, qb, trq[p], trk[p])
            flush_filler()
        for qb in range(4):
            emit_passC_qb(qb)


def _rope_maps():
    """Partition layout within a head-pair tile row block.

    Per head (64 rows): [t1 of pairs 0:16 | t2 of pairs 0:16 |
                         t1 of pairs 16:32 | t2 of pairs 16:32]
    so the rotate-half swap exchanges 16-row blocks within each
    32-partition quadrant (stream_shuffle-expressible).

    Returns (j_idx[128], is_t2[128]) for one 128-row pair tile.
    """
    j_idx = np.zeros(128, np.int64)
    is_t2 = np.zeros(128, bool)
    for p in range(128):
        r = p % 32
        q2 = (p % 64) // 32
        j_idx[p] = q2 * 16 + (r % 16)
        is_t2[p] = r >= 16
    return j_idx, is_t2


def _qk_perm(heads):
    """W-row permutation for one core's 4 pair-tiles (512 rows)."""
    j_idx, is_t2 = _rope_maps()
    rows = []
    for mt in range(4):
        for p in range(128):
            h = heads[2 * mt + p // 64]
            dim = 2 * j_idx[p] + (1 if is_t2[p] else 0)
            rows.append(h * DH + dim)
    return np.array(rows)


def _quant_hi_lo(a, shift):
    s = float(2.0 ** shift)
    hi = np.clip(a * s, -224.0, 224.0).astype(E4)
    lo = (a * s - hi.astype(np.float32)).astype(E4)
    return hi, lo


def prep_inputs(x, WQ, WK, WV, WO, token_positions):
    x = np.asarray(x, np.float32)
    WQ = np.asarray(WQ, np.float32)
    WK = np.asarray(WK, np.float32)
    WV = np.asarray(WV, np.float32)
    WO = np.asarray(WO, np.float32)
    pos = np.asarray(token_positions).astype(np.float32)
    bf = ml_dtypes.bfloat16

    j_idx, is_t2 = _rope_maps()
    invf = (10000.0 ** (-j_idx.astype(np.float32) / 32.0))
    sign = np.where(is_t2, 1.0, -1.0).astype(np.float32)
    ang = pos[None, :] * invf[:, None]
    cosr = (np.cos(ang) * UNDO).astype(np.float32)
    sinr = (np.sin(ang * sign[:, None]) * UNDO).astype(np.float32)

    in_maps = []
    for c in range(NCORE):
        b, g = divmod(c, 2)
        heads = list(range(8 * g, 8 * g + 8))
        perm = _qk_perm(heads)
        rows = slice(8 * g * DH, (8 * g + 8) * DH)

        m = {"cosr": cosr, "sinr": sinr,
             "woT": np.ascontiguousarray(
                 WO.T[:, g * SL:(g + 1) * SL]).astype(bf)}

        # x planes: x8{h,l}{j}[p, plane*2048 + s] = q(x[b, s, (j+4*plane)*128+p])
        xT = x[b].T  # [D, S]
        xh, xl = _quant_hi_lo(xT, XSH)
        for j in range(4):
            m[f"x8h{j}"] = np.ascontiguousarray(
                np.concatenate([xh[j * 128:(j + 1) * 128, :],
                                xh[(j + 4) * 128:(j + 5) * 128, :]], axis=1))
            m[f"x8l{j}"] = np.ascontiguousarray(
                np.concatenate([xl[j * 128:(j + 1) * 128, :],
                                xl[(j + 4) * 128:(j + 5) * 128, :]], axis=1))

        # wq/wk: permuted rows -> [d, m] = W_perm.T; planes along d
        for name, W in (("wq", WQ), ("wk", WK)):
            WpT = np.ascontiguousarray(W[perm, :].T)  # [D, 512]
            wh, wl = _quant_hi_lo(WpT, WSH)
            for j in range(4):
                m[f"{name}8h{j}"] = np.ascontiguousarray(np.concatenate(
                    [wh[j * 128:(j + 1) * 128, :],
                     wh[(j + 4) * 128:(j + 5) * 128, :]], axis=1))
                m[f"{name}8l{j}"] = np.ascontiguousarray(np.concatenate(
                    [wl[j * 128:(j + 1) * 128, :],
                     wl[(j + 4) * 128:(j + 5) * 128, :]], axis=1))

        # wv: [d, m] = WV.T[:, this core's head rows]; planes along d
        WvT = np.ascontiguousarray(WV.T[:, rows])  # [D, 512]
        wh, wl = _quant_hi_lo(WvT, WSH)
        for j in range(4):
            m[f"wv8h{j}"] = np.ascontiguousarray(np.concatenate(
                [wh[j * 128:(j + 1) * 128, :],
                 wh[(j + 4) * 128:(j + 5) * 128, :]], axis=1))
            m[f"wv8l{j}"] = np.ascontiguousarray(np.concatenate(
                [wl[j * 128:(j + 1) * 128, :],
                 wl[(j + 4) * 128:(j + 5) * 128, :]], axis=1))

        in_maps.append(m)
    return in_maps


def assemble(results):
    B = NCORE // 2
    out = np.empty((B, S, D), np.float32)
    for b in range(B):
        out[b, :, 0:SL] = results[2 * b]["out"].astype(np.float32)
        out[b, :, SL:D] = results[2 * b + 1]["out"].astype(np.float32)
    return out


_NC = None


def _get_nc():
    global _NC
    if _NC is None:
        _NC = build()
    return _NC


def kernel(x, WQ, WK, WV, WO, token_positions):
    nc = _get_nc()
    in_maps = prep_inputs(x, WQ, WK, WV, WO, token_positions)
    res = run_bass_kernel_spmd(nc, in_maps, list(range(NCORE)))
    return assemble(res.results)


# revision 68
# speedup vs baseline: 1.1519x; 1.0159x over previous
"""Trainium2 Bass kernel: multi-head flash self-attention with RoPE.

Problem: x[4,2048,1024], 16 heads, dh=64, causal, RoPE(theta=10000), WO proj.

Sharding (8 cores): core c -> batch b=c//2, head-group g=c%2 (8 heads each).

v2 design notes:
  - Q/K/V projections in fp8e4m3 DoubleRow matmuls with a 3-term hi/lo
    split (x_hi*w_hi + x_lo*w_hi + x_hi*w_lo): bf16-level accuracy at
    0.75x the bf16 PE cost. Operands host-prepped in the DoubleRow
    plane-paired layout [d_part, 2, cols] (planes = D-slabs j, j+4),
    scaled by 2^5 (x) and 2^7 (w); the 2^-12 undo is folded into the
    RoPE tables (Q/K) and the V-evacuation copy.
  - RoPE row layout puts each head's pair-halves in 16-row blocks so the
    rotate-half swap is a DVE stream_shuffle (quadrant-local), no DMAs.
    Muls/adds split across DVE and GpSimd.
  - Flash attention in S^T layout ([k,q] blocks), heads A/B fused: scores
    for both heads land in one 2-bank PSUM tile [128,1024]; ONE scalar-
    engine exp per k-block covers both heads (3D access pattern). V is
    stored per (ktile, head-pair) as [V_A | ones | V_B] so softmax
    denominators come out of the PV matmul for free.
  - Per-pair AllGather of normalized O^T (pair 3 exchanged per-qb to
    shorten the tail); output projection accumulates per-pair-group into
    SBUF (passes A/B/C), final store in bf16.
"""
import sys

sys.path.insert(0, "/opt/trn_rl_repo")

import numpy as np
import ml_dtypes
import concourse.bass as bass
import concourse.bacc as bacc
import concourse.mybir as mybir
from concourse import tile
from concourse.bass_utils import run_bass_kernel_spmd

f32 = mybir.dt.float32
bf16 = mybir.dt.bfloat16
fp8 = mybir.dt.float8e4
AF = mybir.ActivationFunctionType
DR = mybir.MatmulPerfMode.DoubleRow
E4 = ml_dtypes.float8_e4m3

S = 2048
D = 1024
H = 16
DH = 64
NCORE = 8
SL = 512           # local m dims (8 heads x 64)
SCALE = 1.0 / 8.0  # 1/sqrt(dh)
GROUPS = [[0, 1], [2, 3], [4, 5], [6, 7]]
VPP = 192          # v_store cols per (ktile, pair): [V_A | ones | V_B]
VKT = 4 * VPP      # v_store cols per ktile
XSH = 5            # x quant scale 2^5
WSH = 7            # w quant scale 2^7
UNDO = 2.0 ** (-(XSH + WSH))
SWAP_MASK = list(range(16, 32)) + list(range(16))  # rotate-half swap


def build(timing=False):
    nc = bacc.Bacc("TRN2", target_bir_lowering=False, debug=False,
                   num_devices=1 if timing else NCORE)

    x8 = {}
    for hl in "hl":
        for j in range(4):
            x8[hl, j] = nc.dram_tensor(f"x8{hl}{j}", [128, 2 * S], fp8,
                                       kind="ExternalInput").ap()
    w8 = {}
    for w in ("wq", "wk", "wv"):
        for hl in "hl":
            for j in range(4):
                w8[w, hl, j] = nc.dram_tensor(f"{w}8{hl}{j}", [128, 1024],
                                              fp8, kind="ExternalInput").ap()
    woT = nc.dram_tensor("woT", [D, SL], bf16, kind="ExternalInput").ap()
    cosr = nc.dram_tensor("cosr", [128, S], f32, kind="ExternalInput").ap()
    sinr = nc.dram_tensor("sinr", [128, S], f32, kind="ExternalInput").ap()
    out = nc.dram_tensor("out", [S, SL], bf16, kind="ExternalOutput").ap()

    og_send = [nc.dram_tensor(f"og_send{p}", [128, S], bf16) for p in range(3)]
    og_recv = [nc.dram_tensor(f"og_recv{p}", [256, S], bf16) for p in range(3)]
    # pair 3 exchanges per-qb; collectives need contiguous dram patterns
    og_send.append([nc.dram_tensor(f"og_send3_{qb}", [128, 512], bf16)
                    for qb in range(4)])
    og_recv.append([nc.dram_tensor(f"og_recv3_{qb}", [256, 512], bf16)
                    for qb in range(4)])

    with tile.TileContext(nc) as tc:
        _body(nc, tc, x8, w8, woT, cosr, sinr, out, og_send, og_recv, timing)
    nc.compile()
    return nc


def _body(nc, tc, x8, w8, woT, cosr, sinr, out, og_send, og_recv,
          timing=False):
    from contextlib import ExitStack
    ctx = ExitStack()
    with ctx:
        sb = ctx.enter_context(tc.tile_pool(name="sb", bufs=1))
        psp = ctx.enter_context(tc.tile_pool(name="psp", bufs=1, space="PSUM"))
        counter = [0]

        def til(shape, dtype, tag, bufs):
            counter[0] += 1
            return sb.tile(shape, dtype, tag=tag, bufs=bufs,
                           name=f"{tag}_{counter[0]}")

        # ---------------- input loads, consumption order ----------------
        # phase 1: x cols 0:512 (both planes) + V weights -> first V quarter
        xt = {}     # (hl, j) -> [128, 4096] fp8 tile, cols = plane*2048 + s
        wvt = {}
        wqt = {}
        wkt = {}

        def x_3d(hl, j):
            return xt[hl, j][:].rearrange("p (j s) -> p j s", j=2)

        # "l"-plane inputs load via the gpsimd SWDGE queue, "h" via SP's
        # HWDGE — halves the serialized per-DMA overhead at startup.
        def eng(hl):
            return nc.sync

        def load_x_phase(ph):
            cols = slice(ph * 512, (ph + 1) * 512)
            for hl in "hl":
                for j in range(4):
                    eng(hl).dma_start(
                        x_3d(hl, j)[:, :, cols],
                        x8[hl, j][:].rearrange("p (j s) -> p j s", j=2)
                        [:, :, cols])

        # gpsimd constants first so they don't queue behind SWDGE loads
        # PE-side causal mask: scores PSUM gets += U^T @ (-BIG*I) on the
        # diagonal 128x128 sub-block, i.e. -1e30 where q < k, so the exp
        # yields exact zeros with no post-exp mask op.
        # U[k, r] = 1 where k < r  (strict lower triangle as lhsT)
        u_t = til([128, 128], bf16, "um", 1)
        nc.gpsimd.memset(u_t[:], 1.0)
        nc.gpsimd.affine_select(
            out=u_t[:], in_=u_t[:], compare_op=mybir.AluOpType.is_gt,
            fill=0.0, base=0, pattern=[[1, 128]], channel_multiplier=-1,
        )
        # IBIG = diag(-1e30)
        ibig_t = til([128, 128], bf16, "ibig", 1)
        nc.gpsimd.memset(ibig_t[:], -1e30)
        nc.gpsimd.affine_select(
            out=ibig_t[:], in_=ibig_t[:], compare_op=mybir.AluOpType.is_ge,
            fill=0.0, base=0, pattern=[[1, 128]], channel_multiplier=-1,
        )
        nc.gpsimd.affine_select(
            out=ibig_t[:], in_=ibig_t[:], compare_op=mybir.AluOpType.is_ge,
            fill=0.0, base=0, pattern=[[-1, 128]], channel_multiplier=1,
        )
        # v quarters: v_q[i] holds ktiles 4i..4i+4; per (kt, pair p) block
        # of VPP cols: [V_A | ones | V_B]
        v_q = []
        for i in range(16):
            vq = til([128, VKT], bf16, "v", 16)
            nc.gpsimd.memset(vq[:], 1.0)
            v_q.append(vq)

        for hl in "hl":
            for j in range(4):
                wvt[hl, j] = til([128, 1024], fp8, "wv", 8)
                xt[hl, j] = til([128, 2 * S], fp8, "x8", 8)
        for j in range(4):
            for hl in "hl":
                eng(hl).dma_start(wvt[hl, j][:], w8["wv", hl, j][:])
                eng(hl).dma_start(
                    x_3d(hl, j)[:, :, 0:512],
                    x8[hl, j][:].rearrange("p (j s) -> p j s", j=2)
                    [:, :, 0:512])
        for j in range(4):
            for hl in "hl":
                wqt[hl, j] = til([128, 1024], fp8, "wq", 8)
                eng(hl).dma_start(wqt[hl, j][:], w8["wq", hl, j][:])
                wkt[hl, j] = til([128, 1024], fp8, "wk", 8)
                eng(hl).dma_start(wkt[hl, j][:], w8["wk", hl, j][:])
        cos_t = til([128, S], f32, "cos", 1)
        nc.sync.dma_start(cos_t[:, 0:512], cosr[:, 0:512])
        sin_t = til([128, S], f32, "sin", 1)
        nc.gpsimd.dma_start(sin_t[:, 0:512], sinr[:, 0:512])
        load_x_phase(1)
        nc.sync.dma_start(cos_t[:, 512:2048], cosr[:, 512:2048])
        nc.gpsimd.dma_start(sin_t[:, 512:2048], sinr[:, 512:2048])
        load_x_phase(2)
        load_x_phase(3)
        wt = []
        for dt in range(8):
            t = til([128, SL], bf16, "wo", 8)
            nc.sync.dma_start(t[:], woT[dt * 128:(dt + 1) * 128, :])
            wt.append(t)

        def proj_mms(ps, stat_of, mov_of):
            """12 DoubleRow matmuls, term-major so they track DMA arrival."""
            terms = [("h", "h"), ("l", "h"), ("h", "l")]
            n = 0
            for (a, b) in terms:
                for j in range(4):
                    n += 1
                    nc.tensor.matmul(
                        ps, stat_of(a, j), mov_of(b, j),
                        start=(n == 1), stop=(n == 12),
                        perf_mode=DR,
                    )

        v_done = [False] * 16

        def emit_v_quarter(i, eager=True):
            for kt4 in range(4):
                kt = 4 * i + kt4
                cell = [None]

                def mk(kt, cell, n, a, b, j):
                    def thunk():
                        if n == 0:
                            counter[0] += 1
                            cell[0] = psp.tile([128, 512], f32,
                                               tag="ps5", bufs=4,
                                               name=f"vps_{counter[0]}")
                        nc.tensor.matmul(
                            cell[0][:],
                            x_3d(a, j)[:, :, kt * 128:(kt + 1) * 128],
                            wvt[b, j][:].rearrange("p (j m) -> p j m", j=2),
                            start=(n == 0), stop=(n == 11), perf_mode=DR)
                        if n == 11:
                            vva = v_q[kt][:].rearrange(
                                "q (a c) -> q a c", c=64)
                            psa = cell[0][:].rearrange(
                                "q (a c) -> q a c", c=64)
                            nc.vector.tensor_scalar_mul(
                                vva[:, 0:12:3, :], psa[:, 0:8:2, :], UNDO)
                            nc.vector.tensor_scalar_mul(
                                vva[:, 2:12:3, :], psa[:, 1:8:2, :], UNDO)
                            v_done[kt] = True
                    return thunk

                terms = [("h", "h"), ("l", "h"), ("h", "l")]
                for n, (a, b, j) in enumerate(
                        (a, b, j) for (a, b) in terms for j in range(4)):
                    t = mk(kt, cell, n, a, b, j)
                    if eager:
                        t()
                    else:
                        filler.append(t)
                if eager:
                    v_done[kt] = True

        def ensure_v(kt):
            while not v_done[kt]:
                filler.popleft()()

        def v_slice(kt, p, c0, c1):
            off = p * VPP
            return v_q[kt][:, off + c0:off + c1]

        from collections import deque
        filler = deque()   # single-MM thunks of attention-independent work

        def pump(n):
            k = 0
            while filler and k < n:
                filler.popleft()()
                k += 1

        def flush_filler():
            while filler:
                filler.popleft()()

        # per-st projection + rope into a [128, 512] bf16 tile. When
        # eager=False the 12 matmuls are enqueued as filler thunks; the
        # rope chain is emitted by the last thunk.
        def proj_rope_st(wtiles, mt, st, eager=True):
            big_t = til([128, 512], bf16, "qk", 17)
            ps = psp.tile([128, 512], f32, tag="ps5", bufs=4)

            def stat(a, j):
                return wtiles[a, j][:].rearrange(
                    "p (j m) -> p j m", j=2)[:, :, mt * 128:(mt + 1) * 128]

            def mov(b, j):
                return x_3d(b, j)[:, :, st * 512:(st + 1) * 512]

            def rope():
                # prefix (eager) runs the whole chain on DVE (Pool busy
                # with SWDGE input loads then); filler ropes split DVE/Pool
                cols = slice(st * 512, (st + 1) * 512)
                tmp = til([128, 512], f32, "tmp", 2)
                nc.vector.tensor_mul(tmp[:], ps[:], cos_t[:, cols])
                swp = til([128, 512], f32, "swp", 2)
                nc.vector.stream_shuffle(swp[:], ps[:], SWAP_MASK)
                swp2 = til([128, 512], f32, "swp2", 2)
                mulv = nc.vector if eager else nc.gpsimd
                mulv.tensor_mul(swp2[:], swp[:], sin_t[:, cols])
                mulv.tensor_add(big_t[:], tmp[:], swp2[:])

            terms = [("h", "h"), ("l", "h"), ("h", "l")]
            steps = [(n, a, b, j) for n, (a, b, j) in enumerate(
                (a, b, j) for (a, b) in terms for j in range(4))]

            def mk(n, a, b, j):
                def thunk():
                    nc.tensor.matmul(ps[:], stat(a, j), mov(b, j),
                                     start=(n == 0), stop=(n == 11),
                                     perf_mode=DR)
                    if n == 11:
                        rope()
                return thunk

            for (n, a, b, j) in steps:
                t = mk(n, a, b, j)
                if eager:
                    t()
                else:
                    filler.append(t)
            return big_t

        ofull = [[None, None] for _ in range(4)]  # [pair][member]

        def exchange_pair(p, qb=None):
            """AllGather pair p's O^T (whole pair, or one qb slice)."""
            if qb is None:
                qcols = slice(0, S)
                snd, rcv = og_send[p][:], og_recv[p]
            else:
                qcols = slice(qb * 512, (qb + 1) * 512)
                snd, rcv = og_send[3][qb][:], og_recv[3][qb]
            if timing:
                # stub the AllGather as two gpsimd-queue (SWDGE) copies,
                # mirroring the real collective's Pool-engine placement
                nc.gpsimd.dma_start(rcv[0:128, :].opt(), snd.opt())
                nc.gpsimd.dma_start(rcv[128:256, :].opt(), snd.opt())
            else:
                nc.gpsimd.collective_compute(
                    "AllGather", mybir.AluOpType.bypass,
                    replica_groups=GROUPS,
                    ins=[snd.opt()], outs=[rcv[:].opt()],
                )
            for g2 in range(2):
                if ofull[p][g2] is None:
                    ofull[p][g2] = til([128, S], bf16, "of", 6)
                nc.sync.dma_start(
                    ofull[p][g2][:, qcols],
                    rcv[g2 * 128:(g2 + 1) * 128, :].opt())

        # -------- per pair: Q/K projection + rope + flash attention --------
        acc = [None] * 16   # SBUF accumulators for the output projection

        def attention_qb(p, qb, qtr, ktr):
            qcols_t = qtr[qb]
            oA = psp.tile([128, 512], f32, tag="ps5", bufs=4)
            oB = psp.tile([128, 512], f32, tag="ps5", bufs=4)
            nkb = 4 * (qb + 1)

            def emit_scores(kb):
                kt_t = ktr[kb // 4]
                kcols = slice((kb % 4) * 128, (kb % 4) * 128 + 128)
                jrel = kb - 4 * qb
                lo = max(jrel, 0) * 128   # first valid q col in block
                sub = slice(lo, 512)
                stAB = psp.tile([128, 1024], f32, tag="st", bufs=2)
                diag = jrel >= 0
                nc.tensor.matmul(stAB[:, lo:512], kt_t[0:64, kcols],
                                 qcols_t[0:64, sub],
                                 start=True, stop=not diag)
                nc.tensor.matmul(stAB[:, 512 + lo:1024],
                                 kt_t[64:128, kcols],
                                 qcols_t[64:128, sub],
                                 start=True, stop=not diag)
                if diag:
                    # accumulate -1e30 on the q<k triangle of the 128-wide
                    # diagonal sub-block (both heads) via the PE
                    nc.tensor.matmul(stAB[:, lo:lo + 128], u_t[:],
                                     ibig_t[:], start=False, stop=True)
                    nc.tensor.matmul(stAB[:, 512 + lo:512 + lo + 128],
                                     u_t[:], ibig_t[:],
                                     start=False, stop=True)
                pAB = til([128, 1024], bf16, "p", 6)
                st3 = stAB[:].rearrange("p (j c) -> p j c", j=2)
                p3 = pAB[:].rearrange("p (j c) -> p j c", j=2)
                nc.scalar.activation(p3[:, :, sub], st3[:, :, sub],
                                     AF.Exp, scale=SCALE)
                return pAB, lo, sub

            def emit_pv(kb, pAB, lo, sub):
                nc.tensor.matmul(oA[:, sub], v_slice(kb, p, 0, 128),
                                 pAB[:, sub],
                                 start=(kb == 0), stop=(kb == nkb - 1))
                nc.tensor.matmul(oB[:, sub], v_slice(kb, p, 64, 192),
                                 pAB[:, 512 + lo:1024],
                                 start=(kb == 0), stop=(kb == nkb - 1))

            # one-block software pipeline: PV(kb) is emitted after
            # scores(kb+1) plus a few filler matmuls, so the exp latency
            # hides behind PE work
            # two-block software pipeline: PV(kb) trails scores(kb+2);
            # PSUM accumulation is order-independent so this is safe, and
            # stAB is freed by the exp, not the PV
            rate = 2 if qb < 3 else 3
            depth = 2
            pend = []
            for kb in range(nkb):
                pend.append((kb,) + emit_scores(kb))
                pump(rate)
                if len(pend) > depth:
                    e = pend.pop(0)
                    if p == 0:
                        ensure_v(e[0])
                    emit_pv(*e)
            while pend:
                pump(rate)
                e = pend.pop(0)
                if p == 0:
                    ensure_v(e[0])
                emit_pv(*e)
            # normalize. A psum rows: [O_A | l_A]; B psum rows: [l_B | O_B]
            qcols = slice(qb * 512, (qb + 1) * 512)
            onrm = til([128, 512], bf16, "onrm", 4)
            rc = til([128, 512], f32, "rc", 2)
            nc.vector.reciprocal(rc[64:128, :], oA[64:128, :])
            nc.vector.reciprocal(rc[0:64, :], oB[0:64, :])
            rc2 = til([128, 512], f32, "rc2", 2)
            nc.sync.dma_start(rc2[0:64, :], rc[64:128, :])
            nc.sync.dma_start(rc2[64:128, :], rc[0:64, :])
            nc.vector.tensor_mul(onrm[0:64, :], oA[0:64, :], rc2[0:64, :])
            nc.vector.tensor_mul(onrm[64:128, :], oB[64:128, :],
                                 rc2[64:128, :])
            if p == 3:
                nc.sync.dma_start(og_send[3][qb][:].opt(), onrm[:])
                exchange_pair(3, qb)
            else:
                nc.sync.dma_start(og_send[p][:, qcols].opt(), onrm[:])

        # output projection passes: A = pairs {0,1}, B = pair 2, C = pair 3
        def out_group(dts, kind, st16, ofin=None):
            cols = slice(st16 * 128, (st16 + 1) * 128)
            cell = [None]

            def tail():
                ps = cell[0]
                if kind == "A":
                    acc[st16] = til([128, SL], bf16, "osb", 16)
                    nc.vector.tensor_copy(acc[st16][:], ps[:])
                elif kind == "B":
                    nc.vector.tensor_add(acc[st16][:], ps[:], acc[st16][:])
                else:
                    nc.vector.tensor_add(
                        ofin[:, (st16 % 4) * 512:(st16 % 4 + 1) * 512],
                        ps[:], acc[st16][:])

            thunks = []
            for i, (p, g2) in enumerate(dts):
                def mk(i, p, g2):
                    def thunk():
                        if i == 0:
                            counter[0] += 1
                            cell[0] = psp.tile([128, 512], f32,
                                               tag="ps5", bufs=4,
                                               name=f"ops_{counter[0]}")
                        nc.tensor.matmul(
                            cell[0][:], ofull[p][g2][:, cols],
                            wt[4 * g2 + p][:],
                            start=(i == 0), stop=(i == len(dts) - 1),
                        )
                        if i == len(dts) - 1:
                            tail()
                    return thunk
                thunks.append(mk(i, p, g2))
            return thunks

        def out_pass(dts, kind, st16s, eager=True):
            for st16 in st16s:
                for t in out_group(dts, kind, st16):
                    if eager:
                        t()
                    else:
                        filler.append(t)

        # ---------------- schedule ----------------
        # p0 prefix: V quarter 0 + p0's Q/K proj+rope eager; V quarters
        # 1-3 go to the filler queue (deadline-guarded by ensure_v)
        trq = {0: [None] * 4}
        trk = {0: [None] * 4}
        for st in range(4):
            emit_v_quarter(st)
            trq[0][st] = proj_rope_st(wqt, 0, st)
            trk[0][st] = proj_rope_st(wkt, 0, st)

        def emit_passC_qb(qb):
            ofin = til([128, 4 * 512], bf16, "ofin", 2)
            for half in range(2):
                for st16 in range(4 * qb + 2 * half, 4 * qb + 2 * half + 2):
                    for t in out_group([(3, 0), (3, 1)], "C", st16,
                                       ofin=ofin):
                        t()
                rows = slice(qb * 512 + half * 256, qb * 512 + half * 256
                             + 256)
                nc.sync.dma_start(
                    out[rows, :].rearrange("(a r) c -> r a c", a=2),
                    ofin[:, half * 1024:(half + 1) * 1024].rearrange(
                        "p (a c) -> p a c", a=2),
                )

        for p in range(4):
            if p > 0:
                exchange_pair(p - 1)
            if p < 3:
                # next pair's Q/K projections as attention filler,
                # interleaved q0,k0,q1,k1,... so early tiles finish first
                trq[p + 1] = [None] * 4
                trk[p + 1] = [None] * 4
                for st in range(4):
                    trq[p + 1][st] = proj_rope_st(wqt, p + 1, st,
                                                  eager=False)
                    trk[p + 1][st] = proj_rope_st(wkt, p + 1, st,
                                                  eager=False)
            else:
                # pass A fills pair 3's attention; pass B is reserved to
                # cover the final exchange chain after the attention
                out_pass([(0, 0), (0, 1), (1, 0), (1, 1)], "A",
                         range(16), eager=False)
            for qb in range(4):
                attention_qb(p, qb, trq[p], trk[p])
            flush_filler()
            if p == 3:
                out_pass([(2, 0), (2, 1)], "B", range(16))
                flush_filler()
        for qb in range(4):
            emit_passC_qb(qb)


def _rope_maps():
    """Partition layout within a head-pair tile row block.

    Per head (64 rows): [t1 of pairs 0:16 | t2 of pairs 0:16 |
                         t1 of pairs 16:32 | t2 of pairs 16:32]
    so the rotate-half swap exchanges 16-row blocks within each
    32-partition quadrant (stream_shuffle-expressible).

    Returns (j_idx[128], is_t2[128]) for one 128-row pair tile.
    """
    j_idx = np.zeros(128, np.int64)
    is_t2 = np.zeros(128, bool)
    for p in range(128):
        r = p % 32
        q2 = (p % 64) // 32
        j_idx[p] = q2 * 16 + (r % 16)
        is_t2[p] = r >= 16
    return j_idx, is_t2


def _qk_perm(heads):
    """W-row permutation for one core's 4 pair-tiles (512 rows)."""
    j_idx, is_t2 = _rope_maps()
    rows = []
    for mt in range(4):
        for p in range(128):
            h = heads[2 * mt + p // 64]
            dim = 2 * j_idx[p] + (1 if is_t2[p] else 0)
            rows.append(h * DH + dim)
    return np.array(rows)


def _quant_hi_lo(a, shift):
    s = float(2.0 ** shift)
    hi = np.clip(a * s, -224.0, 224.0).astype(E4)
    lo = (a * s - hi.astype(np.float32)).astype(E4)
    return hi, lo


def prep_inputs(x, WQ, WK, WV, WO, token_positions):
    x = np.asarray(x, np.float32)
    WQ = np.asarray(WQ, np.float32)
    WK = np.asarray(WK, np.float32)
    WV = np.asarray(WV, np.float32)
    WO = np.asarray(WO, np.float32)
    pos = np.asarray(token_positions).astype(np.float32)
    bf = ml_dtypes.bfloat16

    j_idx, is_t2 = _rope_maps()
    invf = (10000.0 ** (-j_idx.astype(np.float32) / 32.0))
    sign = np.where(is_t2, 1.0, -1.0).astype(np.float32)
    ang = pos[None, :] * invf[:, None]
    cosr = (np.cos(ang) * UNDO).astype(np.float32)
    sinr = (np.sin(ang * sign[:, None]) * UNDO).astype(np.float32)

    in_maps = []
    for c in range(NCORE):
        b, g = divmod(c, 2)
        heads = list(range(8 * g, 8 * g + 8))
        perm = _qk_perm(heads)
        rows = slice(8 * g * DH, (8 * g + 8) * DH)

        m = {"cosr": cosr, "sinr": sinr,
             "woT": np.ascontiguousarray(
                 WO.T[:, g * SL:(g + 1) * SL]).astype(bf)}

        # x planes: x8{h,l}{j}[p, plane*2048 + s] = q(x[b, s, (j+4*plane)*128+p])
        xT = x[b].T  # [D, S]
        xh, xl = _quant_hi_lo(xT, XSH)
        for j in range(4):
            m[f"x8h{j}"] = np.ascontiguousarray(
                np.concatenate([xh[j * 128:(j + 1) * 128, :],
                                xh[(j + 4) * 128:(j + 5) * 128, :]], axis=1))
            m[f"x8l{j}"] = np.ascontiguousarray(
                np.concatenate([xl[j * 128:(j + 1) * 128, :],
                                xl[(j + 4) * 128:(j + 5) * 128, :]], axis=1))

        # wq/wk: permuted rows -> [d, m] = W_perm.T; planes along d
        for name, W in (("wq", WQ), ("wk", WK)):
            WpT = np.ascontiguousarray(W[perm, :].T)  # [D, 512]
            wh, wl = _quant_hi_lo(WpT, WSH)
            for j in range(4):
                m[f"{name}8h{j}"] = np.ascontiguousarray(np.concatenate(
                    [wh[j * 128:(j + 1) * 128, :],
                     wh[(j + 4) * 128:(j + 5) * 128, :]], axis=1))
                m[f"{name}8l{j}"] = np.ascontiguousarray(np.concatenate(
                    [wl[j * 128:(j + 1) * 128, :],
                     wl[(j + 4) * 128:(j + 5) * 128, :]], axis=1))

        # wv: [d, m] = WV.T[:, this core's head rows]; planes along d
        WvT = np.ascontiguousarray(WV.T[:, rows])  # [D, 512]
        wh, wl = _quant_hi_lo(WvT, WSH)
        for j in range(4):
            m[f"wv8h{j}"] = np.ascontiguousarray(np.concatenate(
                [wh[j * 128:(j + 1) * 128, :],
                 wh[(j + 4) * 128:(j + 5) * 128, :]], axis=1))
            m[f"wv8l{j}"] = np.ascontiguousarray(np.concatenate(
                [wl[j * 128:(j + 1) * 128, :],
                 wl[(j + 4) * 128:(j + 5) * 128, :]], axis=1))

        in_maps.append(m)
    return in_maps


def assemble(results):
    B = NCORE // 2
    out = np.empty((B, S, D), np.float32)
    for b in range(B):
        out[b, :, 0:SL] = results[2 * b]["out"].astype(np.float32)
        out[b, :, SL:D] = results[2 * b + 1]["out"].astype(np.float32)
    return out


_NC = None


def _get_nc():
    global _NC
    if _NC is None:
        _NC = build()
    return _NC


def kernel(x, WQ, WK, WV, WO, token_positions):
    nc = _get_nc()
    in_maps = prep_inputs(x, WQ, WK, WV, WO, token_positions)
    res = run_bass_kernel_spmd(nc, in_maps, list(range(NCORE)))
    return assemble(res.results)


# revision 70
# speedup vs baseline: 1.1661x; 1.0124x over previous
"""Trainium2 Bass kernel: multi-head flash self-attention with RoPE.

Problem: x[4,2048,1024], 16 heads, dh=64, causal, RoPE(theta=10000), WO proj.

Sharding (8 cores): core c -> batch b=c//2, head-group g=c%2 (8 heads each).

v2 design notes:
  - Q/K/V projections in fp8e4m3 DoubleRow matmuls with a 3-term hi/lo
    split (x_hi*w_hi + x_lo*w_hi + x_hi*w_lo): bf16-level accuracy at
    0.75x the bf16 PE cost. Operands host-prepped in the DoubleRow
    plane-paired layout [d_part, 2, cols] (planes = D-slabs j, j+4),
    scaled by 2^5 (x) and 2^7 (w); the 2^-12 undo is folded into the
    RoPE tables (Q/K) and the V-evacuation copy.
  - RoPE row layout puts each head's pair-halves in 16-row blocks so the
    rotate-half swap is a DVE stream_shuffle (quadrant-local), no DMAs.
    Muls/adds split across DVE and GpSimd.
  - Flash attention in S^T layout ([k,q] blocks), heads A/B fused: scores
    for both heads land in one 2-bank PSUM tile [128,1024]; ONE scalar-
    engine exp per k-block covers both heads (3D access pattern). V is
    stored per (ktile, head-pair) as [V_A | ones | V_B] so softmax
    denominators come out of the PV matmul for free.
  - Per-pair AllGather of normalized O^T (pair 3 exchanged per-qb to
    shorten the tail); output projection accumulates per-pair-group into
    SBUF (passes A/B/C), final store in bf16.
"""
import sys

sys.path.insert(0, "/opt/trn_rl_repo")

import numpy as np
import ml_dtypes
import concourse.bass as bass
import concourse.bacc as bacc
import concourse.mybir as mybir
from concourse import tile
from concourse.bass_utils import run_bass_kernel_spmd

f32 = mybir.dt.float32
bf16 = mybir.dt.bfloat16
fp8 = mybir.dt.float8e4
AF = mybir.ActivationFunctionType
DR = mybir.MatmulPerfMode.DoubleRow
E4 = ml_dtypes.float8_e4m3

S = 2048
D = 1024
H = 16
DH = 64
NCORE = 8
SL = 512           # local m dims (8 heads x 64)
SCALE = 1.0 / 8.0  # 1/sqrt(dh)
GROUPS = [[0, 1], [2, 3], [4, 5], [6, 7]]
VPP = 192          # v_store cols per (ktile, pair): [V_A | ones | V_B]
VKT = 4 * VPP      # v_store cols per ktile
XSH = 5            # x quant scale 2^5
WSH = 7            # w quant scale 2^7
UNDO = 2.0 ** (-(XSH + WSH))
SWAP_MASK = list(range(16, 32)) + list(range(16))  # rotate-half swap


def build(timing=False):
    nc = bacc.Bacc("TRN2", target_bir_lowering=False, debug=False,
                   num_devices=1 if timing else NCORE)

    x8 = {}
    for hl in "hl":
        for j in range(4):
            x8[hl, j] = nc.dram_tensor(f"x8{hl}{j}", [128, 2 * S], fp8,
                                       kind="ExternalInput").ap()
    w8 = {}
    for w in ("wq", "wk", "wv"):
        for hl in "hl":
            for j in range(4):
                w8[w, hl, j] = nc.dram_tensor(f"{w}8{hl}{j}", [128, 1024],
                                              fp8, kind="ExternalInput").ap()
    woT = nc.dram_tensor("woT", [D, SL], bf16, kind="ExternalInput").ap()
    cosr = nc.dram_tensor("cosr", [128, S], f32, kind="ExternalInput").ap()
    sinr = nc.dram_tensor("sinr", [128, S], f32, kind="ExternalInput").ap()
    out = nc.dram_tensor("out", [S, SL], bf16, kind="ExternalOutput").ap()

    og_send = [nc.dram_tensor(f"og_send{p}", [128, S], bf16) for p in range(3)]
    og_recv = [nc.dram_tensor(f"og_recv{p}", [256, S], bf16) for p in range(3)]
    # pair 3 exchanges per-qb; collectives need contiguous dram patterns
    og_send.append([nc.dram_tensor(f"og_send3_{qb}", [128, 512], bf16)
                    for qb in range(4)])
    og_recv.append([nc.dram_tensor(f"og_recv3_{qb}", [256, 512], bf16)
                    for qb in range(4)])

    with tile.TileContext(nc) as tc:
        _body(nc, tc, x8, w8, woT, cosr, sinr, out, og_send, og_recv, timing)
    nc.compile()
    return nc


def _body(nc, tc, x8, w8, woT, cosr, sinr, out, og_send, og_recv,
          timing=False):
    from contextlib import ExitStack
    ctx = ExitStack()
    with ctx:
        sb = ctx.enter_context(tc.tile_pool(name="sb", bufs=1))
        psp = ctx.enter_context(tc.tile_pool(name="psp", bufs=1, space="PSUM"))
        counter = [0]

        def til(shape, dtype, tag, bufs):
            counter[0] += 1
            return sb.tile(shape, dtype, tag=tag, bufs=bufs,
                           name=f"{tag}_{counter[0]}")

        # ---------------- input loads, consumption order ----------------
        # phase 1: x cols 0:512 (both planes) + V weights -> first V quarter
        xt = {}     # (hl, j) -> [128, 4096] fp8 tile, cols = plane*2048 + s
        wvt = {}
        wqt = {}
        wkt = {}

        def x_3d(hl, j):
            return xt[hl, j][:].rearrange("p (j s) -> p j s", j=2)

        # "l"-plane inputs load via the gpsimd SWDGE queue, "h" via SP's
        # HWDGE — halves the serialized per-DMA overhead at startup.
        def eng(hl):
            return nc.sync

        def load_x_phase(ph):
            cols = slice(ph * 512, (ph + 1) * 512)
            for hl in "hl":
                for j in range(4):
                    eng(hl).dma_start(
                        x_3d(hl, j)[:, :, cols],
                        x8[hl, j][:].rearrange("p (j s) -> p j s", j=2)
                        [:, :, cols])

        # gpsimd constants first so they don't queue behind SWDGE loads
        # PE-side causal mask: scores PSUM gets += U^T @ (-BIG*I) on the
        # diagonal 128x128 sub-block, i.e. -1e30 where q < k, so the exp
        # yields exact zeros with no post-exp mask op.
        # U[k, r] = 1 where k < r  (strict lower triangle as lhsT)
        u_t = til([128, 128], bf16, "um", 1)
        nc.gpsimd.memset(u_t[:], 1.0)
        nc.gpsimd.affine_select(
            out=u_t[:], in_=u_t[:], compare_op=mybir.AluOpType.is_gt,
            fill=0.0, base=0, pattern=[[1, 128]], channel_multiplier=-1,
        )
        # IBIG = diag(-1e30)
        ibig_t = til([128, 128], bf16, "ibig", 1)
        nc.gpsimd.memset(ibig_t[:], -1e30)
        nc.gpsimd.affine_select(
            out=ibig_t[:], in_=ibig_t[:], compare_op=mybir.AluOpType.is_ge,
            fill=0.0, base=0, pattern=[[1, 128]], channel_multiplier=-1,
        )
        nc.gpsimd.affine_select(
            out=ibig_t[:], in_=ibig_t[:], compare_op=mybir.AluOpType.is_ge,
            fill=0.0, base=0, pattern=[[-1, 128]], channel_multiplier=1,
        )
        # v quarters: v_q[i] holds ktiles 4i..4i+4; per (kt, pair p) block
        # of VPP cols: [V_A | ones | V_B]
        v_q = []
        for i in range(16):
            vq = til([128, VKT], bf16, "v", 16)
            nc.gpsimd.memset(vq[:], 1.0)
            v_q.append(vq)

        for hl in "hl":
            for j in range(4):
                wvt[hl, j] = til([128, 1024], fp8, "wv", 8)
                xt[hl, j] = til([128, 2 * S], fp8, "x8", 8)
        for hl in "hl":
            for j in range(4):
                eng(hl).dma_start(wvt[hl, j][:], w8["wv", hl, j][:])
                eng(hl).dma_start(
                    x_3d(hl, j)[:, :, 0:512],
                    x8[hl, j][:].rearrange("p (j s) -> p j s", j=2)
                    [:, :, 0:512])
        for hl in "hl":
            for j in range(4):
                wqt[hl, j] = til([128, 1024], fp8, "wq", 8)
                eng(hl).dma_start(wqt[hl, j][:], w8["wq", hl, j][:])
                wkt[hl, j] = til([128, 1024], fp8, "wk", 8)
                eng(hl).dma_start(wkt[hl, j][:], w8["wk", hl, j][:])
        cos_t = til([128, S], f32, "cos", 1)
        nc.sync.dma_start(cos_t[:, 0:512], cosr[:, 0:512])
        sin_t = til([128, S], f32, "sin", 1)
        nc.gpsimd.dma_start(sin_t[:, 0:512], sinr[:, 0:512])
        load_x_phase(1)
        nc.sync.dma_start(cos_t[:, 512:2048], cosr[:, 512:2048])
        nc.gpsimd.dma_start(sin_t[:, 512:2048], sinr[:, 512:2048])
        load_x_phase(2)
        load_x_phase(3)
        wt = []
        for dt in range(8):
            t = til([128, SL], bf16, "wo", 8)
            nc.sync.dma_start(t[:], woT[dt * 128:(dt + 1) * 128, :])
            wt.append(t)

        def proj_mms(ps, stat_of, mov_of):
            """12 DoubleRow matmuls, term-major so they track DMA arrival."""
            terms = [("h", "h"), ("l", "h"), ("h", "l")]
            n = 0
            for (a, b) in terms:
                for j in range(4):
                    n += 1
                    nc.tensor.matmul(
                        ps, stat_of(a, j), mov_of(b, j),
                        start=(n == 1), stop=(n == 12),
                        perf_mode=DR,
                    )

        v_done = [False] * 16

        def emit_v_quarter(i, eager=True):
            for kt4 in range(4):
                kt = 4 * i + kt4
                cell = [None]

                def mk(kt, cell, n, a, b, j):
                    def thunk():
                        if n == 0:
                            counter[0] += 1
                            cell[0] = psp.tile([128, 512], f32,
                                               tag="ps5", bufs=4,
                                               name=f"vps_{counter[0]}")
                        nc.tensor.matmul(
                            cell[0][:],
                            x_3d(a, j)[:, :, kt * 128:(kt + 1) * 128],
                            wvt[b, j][:].rearrange("p (j m) -> p j m", j=2),
                            start=(n == 0), stop=(n == 11), perf_mode=DR)
                        if n == 11:
                            vva = v_q[kt][:].rearrange(
                                "q (a c) -> q a c", c=64)
                            psa = cell[0][:].rearrange(
                                "q (a c) -> q a c", c=64)
                            nc.vector.tensor_scalar_mul(
                                vva[:, 0:12:3, :], psa[:, 0:8:2, :], UNDO)
                            nc.vector.tensor_scalar_mul(
                                vva[:, 2:12:3, :], psa[:, 1:8:2, :], UNDO)
                            v_done[kt] = True
                    return thunk

                terms = [("h", "h"), ("l", "h"), ("h", "l")]
                for n, (a, b, j) in enumerate(
                        (a, b, j) for (a, b) in terms for j in range(4)):
                    t = mk(kt, cell, n, a, b, j)
                    if eager:
                        t()
                    else:
                        filler.append(t)
                if eager:
                    v_done[kt] = True

        def ensure_v(kt):
            while not v_done[kt]:
                filler.popleft()()

        def v_slice(kt, p, c0, c1):
            off = p * VPP
            return v_q[kt][:, off + c0:off + c1]

        from collections import deque
        filler = deque()   # single-MM thunks of attention-independent work

        def pump(n):
            k = 0
            while filler and k < n:
                filler.popleft()()
                k += 1

        def flush_filler():
            while filler:
                filler.popleft()()

        # per-st projection + rope into a [128, 512] bf16 tile. When
        # eager=False the 12 matmuls are enqueued as filler thunks; the
        # rope chain is emitted by the last thunk.
        def proj_rope_st(wtiles, mt, st, eager=True):
            big_t = til([128, 512], bf16, "qk", 17)
            ps = psp.tile([128, 512], f32, tag="ps5", bufs=4)

            def stat(a, j):
                return wtiles[a, j][:].rearrange(
                    "p (j m) -> p j m", j=2)[:, :, mt * 128:(mt + 1) * 128]

            def mov(b, j):
                return x_3d(b, j)[:, :, st * 512:(st + 1) * 512]

            def rope():
                # prefix (eager) runs the whole chain on DVE (Pool busy
                # with SWDGE input loads then); filler ropes split DVE/Pool
                cols = slice(st * 512, (st + 1) * 512)
                tmp = til([128, 512], f32, "tmp", 2)
                nc.vector.tensor_mul(tmp[:], ps[:], cos_t[:, cols])
                swp = til([128, 512], f32, "swp", 2)
                nc.vector.stream_shuffle(swp[:], ps[:], SWAP_MASK)
                swp2 = til([128, 512], f32, "swp2", 2)
                mulv = nc.vector
                mulv.tensor_mul(swp2[:], swp[:], sin_t[:, cols])
                mulv.tensor_add(big_t[:], tmp[:], swp2[:])

            terms = [("h", "h"), ("l", "h"), ("h", "l")]
            steps = [(n, a, b, j) for n, (a, b, j) in enumerate(
                (a, b, j) for (a, b) in terms for j in range(4))]

            def mk(n, a, b, j):
                def thunk():
                    nc.tensor.matmul(ps[:], stat(a, j), mov(b, j),
                                     start=(n == 0), stop=(n == 11),
                                     perf_mode=DR)
                    if n == 11:
                        rope()
                return thunk

            for (n, a, b, j) in steps:
                t = mk(n, a, b, j)
                if eager:
                    t()
                else:
                    filler.append(t)
            return big_t

        ofull = [[None, None] for _ in range(4)]  # [pair][member]

        def exchange_pair(p, qb=None):
            """AllGather pair p's O^T (whole pair, or one qb slice)."""
            if qb is None:
                qcols = slice(0, S)
                snd, rcv = og_send[p][:], og_recv[p]
            else:
                qcols = slice(qb * 512, (qb + 1) * 512)
                snd, rcv = og_send[3][qb][:], og_recv[3][qb]
            if timing:
                # stub the AllGather as two gpsimd-queue (SWDGE) copies,
                # mirroring the real collective's Pool-engine placement
                nc.gpsimd.dma_start(rcv[0:128, :].opt(), snd.opt())
                nc.gpsimd.dma_start(rcv[128:256, :].opt(), snd.opt())
            else:
                nc.gpsimd.collective_compute(
                    "AllGather", mybir.AluOpType.bypass,
                    replica_groups=GROUPS,
                    ins=[snd.opt()], outs=[rcv[:].opt()],
                )
            for g2 in range(2):
                if ofull[p][g2] is None:
                    ofull[p][g2] = til([128, S], bf16, "of", 6)
                nc.sync.dma_start(
                    ofull[p][g2][:, qcols],
                    rcv[g2 * 128:(g2 + 1) * 128, :].opt())

        # -------- per pair: Q/K projection + rope + flash attention --------
        acc = [None] * 16   # SBUF accumulators for the output projection

        def attention_qb(p, qb, qtr, ktr):
            qcols_t = qtr[qb]
            oA = psp.tile([128, 512], f32, tag="ps5", bufs=4)
            oB = psp.tile([128, 512], f32, tag="ps5", bufs=4)
            nkb = 4 * (qb + 1)

            def emit_scores(kb):
                kt_t = ktr[kb // 4]
                kcols = slice((kb % 4) * 128, (kb % 4) * 128 + 128)
                jrel = kb - 4 * qb
                lo = max(jrel, 0) * 128   # first valid q col in block
                sub = slice(lo, 512)
                stAB = psp.tile([128, 1024], f32, tag="st", bufs=2)
                diag = jrel >= 0
                nc.tensor.matmul(stAB[:, lo:512], kt_t[0:64, kcols],
                                 qcols_t[0:64, sub],
                                 start=True, stop=not diag)
                nc.tensor.matmul(stAB[:, 512 + lo:1024],
                                 kt_t[64:128, kcols],
                                 qcols_t[64:128, sub],
                                 start=True, stop=not diag)
                if diag:
                    # accumulate -1e30 on the q<k triangle of the 128-wide
                    # diagonal sub-block (both heads) via the PE
                    nc.tensor.matmul(stAB[:, lo:lo + 128], u_t[:],
                                     ibig_t[:], start=False, stop=True)
                    nc.tensor.matmul(stAB[:, 512 + lo:512 + lo + 128],
                                     u_t[:], ibig_t[:],
                                     start=False, stop=True)
                pAB = til([128, 1024], bf16, "p", 6)
                st3 = stAB[:].rearrange("p (j c) -> p j c", j=2)
                p3 = pAB[:].rearrange("p (j c) -> p j c", j=2)
                nc.scalar.activation(p3[:, :, sub], st3[:, :, sub],
                                     AF.Exp, scale=SCALE)
                return pAB, lo, sub

            def emit_pv(kb, pAB, lo, sub):
                nc.tensor.matmul(oA[:, sub], v_slice(kb, p, 0, 128),
                                 pAB[:, sub],
                                 start=(kb == 0), stop=(kb == nkb - 1))
                nc.tensor.matmul(oB[:, sub], v_slice(kb, p, 64, 192),
                                 pAB[:, 512 + lo:1024],
                                 start=(kb == 0), stop=(kb == nkb - 1))

            # one-block software pipeline: PV(kb) is emitted after
            # scores(kb+1) plus a few filler matmuls, so the exp latency
            # hides behind PE work
            # two-block software pipeline: PV(kb) trails scores(kb+2);
            # PSUM accumulation is order-independent so this is safe, and
            # stAB is freed by the exp, not the PV
            rate = 2 if qb < 3 else 3
            depth = 2
            pend = []
            for kb in range(nkb):
                pend.append((kb,) + emit_scores(kb))
                pump(rate)
                if len(pend) > depth:
                    e = pend.pop(0)
                    if p == 0:
                        ensure_v(e[0])
                    emit_pv(*e)
            while pend:
                pump(rate)
                e = pend.pop(0)
                if p == 0:
                    ensure_v(e[0])
                emit_pv(*e)
            # normalize. A psum rows: [O_A | l_A]; B psum rows: [l_B | O_B]
            qcols = slice(qb * 512, (qb + 1) * 512)
            onrm = til([128, 512], bf16, "onrm", 4)
            rc = til([128, 512], f32, "rc", 2)
            nc.vector.reciprocal(rc[64:128, :], oA[64:128, :])
            nc.vector.reciprocal(rc[0:64, :], oB[0:64, :])
            rc2 = til([128, 512], f32, "rc2", 2)
            nc.sync.dma_start(rc2[0:64, :], rc[64:128, :])
            nc.sync.dma_start(rc2[64:128, :], rc[0:64, :])
            nc.vector.tensor_mul(onrm[0:64, :], oA[0:64, :], rc2[0:64, :])
            nc.vector.tensor_mul(onrm[64:128, :], oB[64:128, :],
                                 rc2[64:128, :])
            if p == 3:
                nc.sync.dma_start(og_send[3][qb][:].opt(), onrm[:])
                exchange_pair(3, qb)
            else:
                nc.sync.dma_start(og_send[p][:, qcols].opt(), onrm[:])

        # output projection passes: A = pairs {0,1}, B = pair 2, C = pair 3
        def out_group(dts, kind, st16, ofin=None):
            cols = slice(st16 * 128, (st16 + 1) * 128)
            cell = [None]

            def tail():
                ps = cell[0]
                if kind == "A":
                    acc[st16] = til([128, SL], bf16, "osb", 16)
                    nc.vector.tensor_copy(acc[st16][:], ps[:])
                elif kind == "B":
                    nc.vector.tensor_add(acc[st16][:], ps[:], acc[st16][:])
                else:
                    nc.vector.tensor_add(
                        ofin[:, (st16 % 4) * 512:(st16 % 4 + 1) * 512],
                        ps[:], acc[st16][:])

            thunks = []
            for i, (p, g2) in enumerate(dts):
                def mk(i, p, g2):
                    def thunk():
                        if i == 0:
                            counter[0] += 1
                            cell[0] = psp.tile([128, 512], f32,
                                               tag="ps5", bufs=4,
                                               name=f"ops_{counter[0]}")
                        nc.tensor.matmul(
                            cell[0][:], ofull[p][g2][:, cols],
                            wt[4 * g2 + p][:],
                            start=(i == 0), stop=(i == len(dts) - 1),
                        )
                        if i == len(dts) - 1:
                            tail()
                    return thunk
                thunks.append(mk(i, p, g2))
            return thunks

        def out_pass(dts, kind, st16s, eager=True):
            for st16 in st16s:
                for t in out_group(dts, kind, st16):
                    if eager:
                        t()
                    else:
                        filler.append(t)

        # ---------------- schedule ----------------
        # p0 prefix: V quarter 0 + p0's Q/K proj+rope eager; V quarters
        # 1-3 go to the filler queue (deadline-guarded by ensure_v)
        trq = {0: [None] * 4}
        trk = {0: [None] * 4}
        for st in range(4):
            emit_v_quarter(st)
            trq[0][st] = proj_rope_st(wqt, 0, st)
            trk[0][st] = proj_rope_st(wkt, 0, st)

        def emit_passC_qb(qb):
            ofin = til([128, 4 * 512], bf16, "ofin", 2)
            for half in range(2):
                for st16 in range(4 * qb + 2 * half, 4 * qb + 2 * half + 2):
                    for t in out_group([(3, 0), (3, 1)], "C", st16,
                                       ofin=ofin):
                        t()
                rows = slice(qb * 512 + half * 256, qb * 512 + half * 256
                             + 256)
                nc.sync.dma_start(
                    out[rows, :].rearrange("(a r) c -> r a c", a=2),
                    ofin[:, half * 1024:(half + 1) * 1024].rearrange(
                        "p (a c) -> p a c", a=2),
                )

        for p in range(4):
            if p > 0:
                exchange_pair(p - 1)
            if p < 3:
                # next pair's Q/K projections as attention filler,
                # interleaved q0,k0,q1,k1,... so early tiles finish first
                trq[p + 1] = [None] * 4
                trk[p + 1] = [None] * 4
                for st in range(4):
                    trq[p + 1][st] = proj_rope_st(wqt, p + 1, st,
                                                  eager=False)
                    trk[p + 1][st] = proj_rope_st(wkt, p + 1, st,
                                                  eager=False)
            else:
                # pass A fills pair 3's attention; pass B is reserved to
                # cover the final exchange chain after the attention
                out_pass([(0, 0), (0, 1), (1, 0), (1, 1)], "A",
                         range(16), eager=False)
            for qb in range(4):
                attention_qb(p, qb, trq[p], trk[p])
            flush_filler()
            if p == 3:
                out_pass([(2, 0), (2, 1)], "B", range(16))
                flush_filler()
        for qb in range(4):
            emit_passC_qb(qb)


def _rope_maps():
    """Partition layout within a head-pair tile row block.

    Per head (64 rows): [t1 of pairs 0:16 | t2 of pairs 0:16 |
                         t1 of pairs 16:32 | t2 of pairs 16:32]
    so the rotate-half swap exchanges 16-row blocks within each
    32-partition quadrant (stream_shuffle-expressible).

    Returns (j_idx[128], is_t2[128]) for one 128-row pair tile.
    """
    j_idx = np.zeros(128, np.int64)
    is_t2 = np.zeros(128, bool)
    for p in range(128):
        r = p % 32
        q2 = (p % 64) // 32
        j_idx[p] = q2 * 16 + (r % 16)
        is_t2[p] = r >= 16
    return j_idx, is_t2


def _qk_perm(heads):
    """W-row permutation for one core's 4 pair-tiles (512 rows)."""
    j_idx, is_t2 = _rope_maps()
    rows = []
    for mt in range(4):
        for p in range(128):
            h = heads[2 * mt + p // 64]
            dim = 2 * j_idx[p] + (1 if is_t2[p] else 0)
            rows.append(h * DH + dim)
    return np.array(rows)


def _quant_hi_lo(a, shift):
    s = float(2.0 ** shift)
    hi = np.clip(a * s, -224.0, 224.0).astype(E4)
    lo = (a * s - hi.astype(np.float32)).astype(E4)
    return hi, lo


def prep_inputs(x, WQ, WK, WV, WO, token_positions):
    x = np.asarray(x, np.float32)
    WQ = np.asarray(WQ, np.float32)
    WK = np.asarray(WK, np.float32)
    WV = np.asarray(WV, np.float32)
    WO = np.asarray(WO, np.float32)
    pos = np.asarray(token_positions).astype(np.float32)
    bf = ml_dtypes.bfloat16

    j_idx, is_t2 = _rope_maps()
    invf = (10000.0 ** (-j_idx.astype(np.float32) / 32.0))
    sign = np.where(is_t2, 1.0, -1.0).astype(np.float32)
    ang = pos[None, :] * invf[:, None]
    cosr = (np.cos(ang) * UNDO).astype(np.float32)
    sinr = (np.sin(ang * sign[:, None]) * UNDO).astype(np.float32)

    in_maps = []
    for c in range(NCORE):
        b, g = divmod(c, 2)
        heads = list(range(8 * g, 8 * g + 8))
        perm = _qk_perm(heads)
        rows = slice(8 * g * DH, (8 * g + 8) * DH)

        m = {"cosr": cosr, "sinr": sinr,
             "woT": np.ascontiguousarray(
                 WO.T[:, g * SL:(g + 1) * SL]).astype(bf)}

        # x planes: x8{h,l}{j}[p, plane*2048 + s] = q(x[b, s, (j+4*plane)*128+p])
        xT = x[b].T  # [D, S]
        xh, xl = _quant_hi_lo(xT, XSH)
        for j in range(4):
            m[f"x8h{j}"] = np.ascontiguousarray(
                np.concatenate([xh[j * 128:(j + 1) * 128, :],
                                xh[(j + 4) * 128:(j + 5) * 128, :]], axis=1))
            m[f"x8l{j}"] = np.ascontiguousarray(
                np.concatenate([xl[j * 128:(j + 1) * 128, :],
                                xl[(j + 4) * 128:(j + 5) * 128, :]], axis=1))

        # wq/wk: permuted rows -> [d, m] = W_perm.T; planes along d
        for name, W in (("wq", WQ), ("wk", WK)):
            WpT = np.ascontiguousarray(W[perm, :].T)  # [D, 512]
            wh, wl = _quant_hi_lo(WpT, WSH)
            for j in range(4):
                m[f"{name}8h{j}"] = np.ascontiguousarray(np.concatenate(
                    [wh[j * 128:(j + 1) * 128, :],
                     wh[(j + 4) * 128:(j + 5) * 128, :]], axis=1))
                m[f"{name}8l{j}"] = np.ascontiguousarray(np.concatenate(
                    [wl[j * 128:(j + 1) * 128, :],
                     wl[(j + 4) * 128:(j + 5) * 128, :]], axis=1))

        # wv: [d, m] = WV.T[:, this core's head rows]; planes along d
        WvT = np.ascontiguousarray(WV.T[:, rows])  # [D, 512]
        wh, wl = _quant_hi_lo(WvT, WSH)
        for j in range(4):
            m[f"wv8h{j}"] = np.ascontiguousarray(np.concatenate(
                [wh[j * 128:(j + 1) * 128, :],
                 wh[(j + 4) * 128:(j + 5) * 128, :]], axis=1))
            m[f"wv8l{j}"] = np.ascontiguousarray(np.concatenate(
                [wl[j * 128:(j + 1) * 128, :],
                 wl[(j + 4) * 128:(j + 5) * 128, :]], axis=1))

        in_maps.append(m)
    return in_maps


def assemble(results):
    B = NCORE // 2
    out = np.empty((B, S, D), np.float32)
    for b in range(B):
        out[b, :, 0:SL] = results[2 * b]["out"].astype(np.float32)
        out[b, :, SL:D] = results[2 * b + 1]["out"].astype(np.float32)
    return out


_NC = None


def _get_nc():
    global _NC
    if _NC is None:
        _NC = build()
    return _NC


def kernel(x, WQ, WK, WV, WO, token_positions):
    nc = _get_nc()
    in_maps = prep_inputs(x, WQ, WK, WV, WO, token_positions)
    res = run_bass_kernel_spmd(nc, in_maps, list(range(NCORE)))
    return assemble(res.results)
